# revision 1
# baseline (speedup 1.0000x reference)
"""Trainium2 Bass kernel for nn_NetFV (NetFV pooling head).

Strategy (pure data parallel over 8 cores, 256 batches each):
  - Host: cast x to bf16 twice: natural [B*M, F] and transposed-augmented
    [B, F+1, 608] (row F = ones -> bias fold; cols 600:608 zero pad so every
    128-wide chunk window is in-bounds). Also pre-fold all per-(f,c) finishing
    constants and rearrange the head weights.
  - Device, per superbatch of 8 batches (40 chunks of 120 rows):
      logits chunk [120,8] = matmul(lhsT=XT[61,128] (FWL), rhs=Waug[61,8])
      softmax: one exp / rowsum / recip / mul over the whole superbatch
      fv chunk: matmul(lhsT=Xgrp[120,128]=(x|x^2|ones|pad) (FWL),
                       rhs=act[120,8]) accumulated over 5 chunks per batch
      -> psum [128(=fv1|fv2|asum|junk), 8] per batch
  - Finishing per 64 batches, f-on-partitions: all elementwise work as
    [60,512] DVE ops with const broadcast APs; partition reductions and
    partition broadcasts via tiny PE matmuls with ones vectors; second
    l2_normalize of fv1 folded as 1/sqrt(C) into the head weights; head as
    16 accumulated [60,64]x[60,18] matmuls.
"""

import math
import sys

for _p in ("/opt/trn_rl_repo", "/opt/pypackages"):
    if _p not in sys.path:
        sys.path.append(_p)

import ml_dtypes
import numpy as np

import concourse.bacc as bacc
import concourse.bass as bass
import concourse.mybir as mybir
import concourse.tile as tile
from concourse.bass_utils import run_bass_kernel_spmd

F, M, C, OUT = 60, 600, 8, 18
B = 2048
NCORES = 8
BL = B // NCORES            # 256 batches per core
SB = 8                      # batches per superbatch
NSB = BL // SB              # 32 superbatches
FGB = 64                    # batches per finishing group
NFG = BL // FGB             # 4 finishing groups
SBPF = FGB // SB            # 8 superbatches per finishing group
CH = 5                      # chunks (of 120 rows) per batch
RP = M // CH                # 120 rows per chunk
XTW = 608                   # padded transposed row length
NG = FGB * C                # 512 finishing columns

BF16 = mybir.dt.bfloat16
F32 = mybir.dt.float32
MULT = mybir.AluOpType.mult
EPS = 1e-12


def _build_nc():
    nc = bacc.Bacc(
        "TRN2", target_bir_lowering=False, debug=False,
        enable_asserts=False, num_devices=NCORES,
    )
    # superbatch-packed layouts: one contiguous run per partition per DMA
    xg = nc.dram_tensor("xg", [NSB, RP, SB * CH * F], BF16,
                        kind="ExternalInput").ap()
    xt = nc.dram_tensor("xt", [NSB, F + 1, SB * XTW], BF16,
                        kind="ExternalInput").ap()
    waug_d = nc.dram_tensor("waug", [F + 1, C], BF16, kind="ExternalInput").ap()
    cst_d = nc.dram_tensor("cst", [128, 5 * C], F32, kind="ExternalInput").ap()
    hds_d = nc.dram_tensor("hds", [F, 2 * C * OUT], F32, kind="ExternalInput").ap()
    y = nc.dram_tensor("y", [BL, OUT], F32, kind="ExternalOutput").ap()

    with tile.TileContext(nc) as tc:
        _emit(tc, y, xg, xt, waug_d, cst_d, hds_d)
    nc.compile()
    return nc


def _emit(tc, y, xg, xt, waug_d, cst_d, hds_d):
    nc = tc.nc
    from contextlib import ExitStack
    ctx = ExitStack()
    with ctx:
        cpool = ctx.enter_context(tc.tile_pool(name="cpool", bufs=1))
        xpool = ctx.enter_context(tc.tile_pool(name="xpool", bufs=3))
        tpool = ctx.enter_context(tc.tile_pool(name="tpool", bufs=3))
        spool = ctx.enter_context(tc.tile_pool(name="spool", bufs=3))
        gpool = ctx.enter_context(tc.tile_pool(name="gpool", bufs=2))
        fpool = ctx.enter_context(tc.tile_pool(name="fpool", bufs=2))
        lpsum = ctx.enter_context(tc.tile_pool(name="lpsum", bufs=2, space="PSUM"))
        fpsum = ctx.enter_context(tc.tile_pool(name="fpsum", bufs=2, space="PSUM"))
        apsum = ctx.enter_context(tc.tile_pool(name="apsum", bufs=1, space="PSUM"))
        bpsum = ctx.enter_context(tc.tile_pool(name="bpsum", bufs=1, space="PSUM"))
        rpsum = ctx.enter_context(tc.tile_pool(name="rpsum", bufs=1, space="PSUM"))
        npsum = ctx.enter_context(tc.tile_pool(name="npsum", bufs=1, space="PSUM"))

        # ---- constants ----
        waug = cpool.tile([F + 1, C], BF16)
        nc.sync.dma_start(out=waug[:], in_=waug_d[:])
        cst = cpool.tile([128, 5 * C], F32)
        nc.sync.dma_start(out=cst[:], in_=cst_d[:])
        hds = cpool.tile([F, 2 * C * OUT], F32)
        nc.sync.dma_start(out=hds[:], in_=hds_d[:])
        k1 = cst[0:F, 0 * C:1 * C]
        w2k1 = cst[0:F, 1 * C:2 * C]
        bco64 = cst[64:64 + F, 2 * C:3 * C]   # used against stage[64:124]
        cco = cst[0:F, 3 * C:4 * C]
        dco = cst[0:F, 4 * C:5 * C]

        ones_r = cpool.tile([1, F], F32)   # lhsT for partition-broadcast
        nc.vector.memset(ones_r[:], 1.0)
        ones_c = cpool.tile([F, 1], F32)   # lhsT for partition-sum
        nc.vector.memset(ones_c[:], 1.0)
        eps1 = cpool.tile([1, 1], F32)     # l2-normalize epsilon
        nc.vector.memset(eps1[:], EPS)
        onem = cpool.tile([RP, 1], BF16)   # ones column: asum matmuls
        nc.vector.memset(onem[:], 1.0)

        def cb(ap):  # broadcast a [F, C] const across FGB batches
            return ap.unsqueeze(1).broadcast_to([F, FGB, C])

        for fg in range(NFG):
            stage = gpool.tile([128, NG], F32)
            asumst = gpool.tile([1, NG], F32)
            for s in range(SBPF):
                sb = fg * SBPF + s
                # ---- load superbatch ----
                xgt = xpool.tile([RP, SB * CH * 128], BF16)
                nc.sync.dma_start(
                    out=xgt.rearrange("p (k q) -> p k q", q=128)[:, :, 0:F],
                    in_=xg[sb].rearrange("p (k f) -> p k f", f=F),
                )
                # x^2 into cols 64:124 (32-aligned partition base after matmul)
                xgv = xgt.rearrange("p (k q) -> p k q", q=128)
                nc.vector.tensor_mul(
                    xgv[:, :, 64:64 + F], xgv[:, :, 0:F], xgv[:, :, 0:F]
                )
                xtt = tpool.tile([F + 1, SB * XTW], BF16)
                nc.sync.dma_start(out=xtt[:], in_=xt[sb])
                # ---- logits ----
                lp = lpsum.tile([128, SB * CH * C], F32)
                for b in range(SB):
                    for c in range(CH):
                        nc.tensor.matmul(
                            lp[:, (b * CH + c) * C:(b * CH + c + 1) * C],
                            xtt[:, b * XTW + c * RP: b * XTW + c * RP + 128],
                            waug[:],
                            start=True, stop=True,
                        )
                # ---- softmax over C ----
                expt = spool.tile([RP, SB * CH * C], F32, tag="expt")
                nc.scalar.activation(
                    expt[:], lp[0:RP, :], mybir.ActivationFunctionType.Exp
                )
                sums = spool.tile([RP, SB * CH], F32, tag="sums")
                nc.vector.reduce_sum(
                    out=sums[:],
                    in_=expt.rearrange("p (k e) -> p k e", e=C),
                    axis=mybir.AxisListType.X,
                )
                rin = spool.tile([RP, SB * CH], F32, tag="rin")
                nc.vector.reciprocal(rin[:], sums[:])
                actt = spool.tile([RP, SB * CH * C], BF16, tag="actt")
                nc.vector.tensor_tensor(
                    out=actt.rearrange("p (k e) -> p k e", e=C),
                    in0=expt.rearrange("p (k e) -> p k e", e=C),
                    in1=rin.unsqueeze(2).broadcast_to([RP, SB * CH, C]),
                    op=MULT,
                )
                # ---- fv accumulation ----
                fp = fpsum.tile([128, SB * C], F32)
                ap_ = apsum.tile([1, SB * C], F32)
                for b in range(SB):
                    for c in range(CH):
                        nc.tensor.matmul(
                            fp[:, b * C:(b + 1) * C],
                            xgt[:, (b * CH + c) * 128:(b * CH + c + 1) * 128],
                            actt[:, (b * CH + c) * C:(b * CH + c + 1) * C],
                            start=(c == 0), stop=(c == CH - 1),
                        )
                        nc.tensor.matmul(
                            ap_[:, b * C:(b + 1) * C],
                            onem[:],
                            actt[:, (b * CH + c) * C:(b * CH + c + 1) * C],
                            start=(c == 0), stop=(c == CH - 1),
                        )
                nc.vector.tensor_copy(
                    stage[:, s * SB * C:(s + 1) * SB * C], fp[:]
                )
                nc.scalar.copy(
                    asumst[:, s * SB * C:(s + 1) * SB * C], ap_[:]
                )

            # ---- finishing for this group of 64 batches ----
            fv1r = stage[0:F, :]
            fv2r = stage[64:64 + F, :]
            asb = bpsum.tile([F, NG], F32)
            nc.tensor.matmul(asb[:], ones_r[:], asumst[:], start=True, stop=True)

            t1 = fpool.tile([F, NG], F32, tag="t1")
            nc.vector.tensor_tensor(out=t1.rearrange("p (g e) -> p g e", e=C),
                                    in0=fv1r.rearrange("p (g e) -> p g e", e=C),
                                    in1=cb(k1), op=MULT)
            m1 = fpool.tile([F, NG], F32, tag="m1")
            nc.vector.tensor_tensor(out=m1.rearrange("p (g e) -> p g e", e=C),
                                    in0=asb.rearrange("p (g e) -> p g e", e=C),
                                    in1=cb(w2k1), op=MULT)
            fv1f = fpool.tile([F, NG], F32, tag="fv1f")
            nc.vector.tensor_sub(fv1f[:], t1[:], m1[:])
            q1 = fpool.tile([F, NG], F32, tag="q1")
            nc.vector.tensor_mul(q1[:], fv1f[:], fv1f[:])
            r1 = rpsum.tile([1, NG], F32, tag="rs")
            nc.tensor.matmul(r1[:], ones_c[:], q1[:], start=True, stop=True)
            sq1 = fpool.tile([1, NG], F32, tag="sq1")
            nc.scalar.activation(sq1[:], r1[:],
                                 mybir.ActivationFunctionType.Sqrt, bias=eps1[:])
            nr1 = fpool.tile([1, NG], F32, tag="nr1")
            nc.vector.reciprocal(nr1[:], sq1[:])
            nb1 = npsum.tile([F, NG], F32, tag="nb")
            nc.tensor.matmul(nb1[:], ones_r[:], nr1[:], start=True, stop=True)
            fv1n = fpool.tile([F, NG], F32, tag="fv1n")
            nc.vector.tensor_mul(fv1n[:], fv1f[:], nb1[:])

            u1 = fpool.tile([F, NG], F32, tag="u1")
            nc.vector.tensor_tensor(out=u1.rearrange("p (g e) -> p g e", e=C),
                                    in0=asb.rearrange("p (g e) -> p g e", e=C),
                                    in1=cb(dco), op=MULT)
            u2 = fpool.tile([F, NG], F32, tag="u2")
            nc.vector.tensor_tensor(out=u2.rearrange("p (g e) -> p g e", e=C),
                                    in0=fv2r.rearrange("p (g e) -> p g e", e=C),
                                    in1=bco64.unsqueeze(1).broadcast_to([F, FGB, C]),
                                    op=MULT)
            u3 = fpool.tile([F, NG], F32, tag="u3")
            nc.vector.tensor_add(u3[:], u1[:], u2[:])
            u4 = fpool.tile([F, NG], F32, tag="u4")
            nc.vector.tensor_tensor(out=u4.rearrange("p (g e) -> p g e", e=C),
                                    in0=fv1r.rearrange("p (g e) -> p g e", e=C),
                                    in1=cb(cco), op=MULT)
            fv2n = fpool.tile([F, NG], F32, tag="fv2n")
            nc.vector.tensor_sub(fv2n[:], u3[:], u4[:])
            q2 = fpool.tile([F, NG], F32, tag="q2")
            nc.vector.tensor_mul(q2[:], fv2n[:], fv2n[:])
            r2 = rpsum.tile([1, NG], F32, tag="rs")
            nc.tensor.matmul(r2[:], ones_c[:], q2[:], start=True, stop=True)
            r2c = fpool.tile([1, FGB], F32, tag="r2c")
            nc.vector.reduce_sum(out=r2c[:],
                                 in_=r2.rearrange("p (g e) -> p g e", e=C),
                                 axis=mybir.AxisListType.X)
            sq2 = fpool.tile([1, FGB], F32, tag="sq2")
            nc.scalar.activation(sq2[:], r2c[:],
                                 mybir.ActivationFunctionType.Sqrt, bias=eps1[:])
            nr2 = fpool.tile([1, FGB], F32, tag="nr2")
            nc.vector.reciprocal(nr2[:], sq2[:])
            nr2e = fpool.tile([1, NG], F32, tag="nr2e")
            nc.vector.tensor_copy(
                nr2e.rearrange("p (g e) -> p g e", e=C),
                nr2.unsqueeze(2).broadcast_to([1, FGB, C]),
            )
            nb2 = npsum.tile([F, NG], F32, tag="nb")
            nc.tensor.matmul(nb2[:], ones_r[:], nr2e[:], start=True, stop=True)
            fv2nn = fpool.tile([F, NG], F32, tag="fv2nn")
            nc.vector.tensor_mul(fv2nn[:], fv2n[:], nb2[:])

            # ---- head ----
            hp = rpsum.tile([FGB, OUT], F32, tag="rs")
            for ci in range(C):
                nc.tensor.matmul(
                    hp[:], fv1n[:, ci::C], hds[:, ci * OUT:(ci + 1) * OUT],
                    start=(ci == 0), stop=False,
                )
            for ci in range(C):
                nc.tensor.matmul(
                    hp[:], fv2nn[:, ci::C],
                    hds[:, (C + ci) * OUT:(C + ci + 1) * OUT],
                    start=False, stop=(ci == C - 1),
                )
            yt = fpool.tile([FGB, OUT], F32, tag="yt")
            nc.scalar.copy(yt[:], hp[:])
            nc.sync.dma_start(out=y[fg * FGB:(fg + 1) * FGB, :], in_=yt[:])


def _host_prep(reshaped_input, cluster_weights, covar_weights, cluster_biases,
               cluster_weights2, hidden1_weights):
    bf = ml_dtypes.bfloat16
    x = np.ascontiguousarray(reshaped_input, dtype=np.float32)
    xb = x.astype(bf)                                   # [B*M, F]
    x3 = xb.reshape(B, M, F)
    xtr = np.zeros((B, F + 1, XTW), dtype=bf)
    xtr[:, :F, :M] = x3.transpose(0, 2, 1)
    xtr[:, F, :M] = bf(1.0)
    # superbatch-packed: xgp[core][sb, p, k*F+f], xtp[core][sb, :, b*XTW+q]
    xgp = (xb.reshape(NCORES, NSB, SB * CH, RP, F)
             .transpose(0, 1, 3, 2, 4)
             .reshape(NCORES, NSB, RP, SB * CH * F))
    xtp = (xtr.reshape(NCORES, NSB, SB, F + 1, XTW)
              .transpose(0, 1, 3, 2, 4)
              .reshape(NCORES, NSB, F + 1, SB * XTW))

    waug = np.concatenate(
        [cluster_weights, cluster_biases[None, :]], axis=0
    ).astype(bf)                                        # [61, 8]

    cw = np.square(covar_weights.astype(np.float64)) + 1e-6       # [F, C]
    w2 = cluster_weights2[0].astype(np.float64)                   # [F, C]
    k1 = 1.0 / cw
    w2k1 = w2 / cw
    bcc = 1.0 / np.square(cw)
    ccc = 2.0 * w2 / np.square(cw)
    dcc = np.square(w2) / np.square(cw) - 1.0
    cst60 = np.concatenate([k1, w2k1, bcc, ccc, dcc], axis=1).astype(np.float32)
    cst = np.zeros((128, 5 * C), dtype=np.float32)
    cst[0:F] = cst60
    cst[64:64 + F] = cst60

    h = hidden1_weights.astype(np.float64)              # [2*C*F, OUT]
    h1 = h[:C * F].reshape(F, C, OUT) / math.sqrt(C)    # fold 2nd l2n of fv1
    h2 = h[C * F:].reshape(F, C, OUT)
    hds = np.concatenate([h1, h2], axis=1).reshape(F, 2 * C * OUT)
    hds = np.ascontiguousarray(hds, dtype=np.float32)

    in_maps = []
    for ci in range(NCORES):
        in_maps.append({
            "xg": np.ascontiguousarray(xgp[ci]),
            "xt": np.ascontiguousarray(xtp[ci]),
            "waug": waug,
            "cst": cst,
            "hds": hds,
        })
    return in_maps


_CACHE = {}


def _get_nc():
    if "nc" not in _CACHE:
        _CACHE["nc"] = _build_nc()
    return _CACHE["nc"]


def kernel(reshaped_input, cluster_weights, covar_weights, cluster_biases,
           cluster_weights2, hidden1_weights, **_kw):
    in_maps = _host_prep(reshaped_input, cluster_weights, covar_weights,
                         cluster_biases, cluster_weights2, hidden1_weights)
    nc = _get_nc()
    res = run_bass_kernel_spmd(nc, in_maps, list(range(NCORES)))
    ys = [res.results[ci]["y"] for ci in range(NCORES)]
    return np.ascontiguousarray(np.concatenate(ys, axis=0), dtype=np.float32)


if __name__ == "__main__":
    rng = np.random.default_rng(0)
    fake = {
        "reshaped_input": rng.standard_normal((B * M, F), dtype=np.float32),
        "cluster_weights": rng.standard_normal((F, C)).astype(np.float32) * 0.13,
        "covar_weights": rng.standard_normal((F, C)).astype(np.float32) * 0.13,
        "cluster_biases": rng.standard_normal((C,)).astype(np.float32) * 0.13,
        "cluster_weights2": rng.standard_normal((1, F, C)).astype(np.float32) * 0.13,
        "hidden1_weights": rng.standard_normal((2 * C * F, OUT)).astype(np.float32) * 0.35,
    }
    out = kernel(**fake)
    print("kernel output", out.shape, out.dtype, np.abs(out).mean())



# revision 3
# speedup vs baseline: 2.3295x; 2.3295x over previous
"""Trainium2 Bass kernel for nn_NetFV (NetFV pooling head).

Strategy (pure data parallel over 8 cores, 256 batches each):
  - Host: cast x to bf16 in two layouts, both DMA'd as fully contiguous
    ~4.8KB-per-partition lines (this is the whole ballgame: the kernel is
    HBM-bound and small descriptors halve-or-worse the DMA bus):
      xg  [NSB, 120, 2400]   m-major: row p = sample-within-chunk, cols
                             (b*5+c)*60+f for superbatch-batch b, chunk c.
      xt2 [NSB, 121, 2432]   f-major batch-PAIR packed: rows 0:60 = batch
                             even's 60 features, rows 60:120 = batch odd,
                             row 120 = ones (bias fold); cols pr*608+m with
                             m 600:608 zero-padded so every 128-wide matmul
                             window is in-bounds. 121/128 partitions vs the
                             naive 61/128.
  - Device, per superbatch of 8 batches:
      logits: 20 matmuls lhsT=xt2[121,128] window, rhs=block-diag W [121,16]
              -> psum [128, 16] (two batches at once)
      softmax: exp (Act engine), rowsum/recip/mul (DVE) on [120, 320]
      x^2: one DVE square [120, 2400] into its own tile
      fv: per batch per chunk, 3 accumulating weight-stationary matmuls
          (lhsT = x-chunk [120,60] -> psum rows 0:60; x^2-chunk -> rows
          64:124; ones [120,1] -> a_sum), each out free-size 8 (cheap).
  - Finishing per 64 batches, f-on-partitions [60, 512]: elementwise DVE ops
    with folded constants; partition reductions/broadcasts via tiny PE
    matmuls; second l2-normalize of fv1 folded into head weights; head as 16
    accumulated [60,64]x[60,18] matmuls.
"""

import math
import sys

for _p in ("/opt/trn_rl_repo", "/opt/pypackages"):
    if _p not in sys.path:
        sys.path.append(_p)

import ml_dtypes
import numpy as np

import concourse.bacc as bacc
import concourse.bass as bass
import concourse.mybir as mybir
import concourse.tile as tile
from concourse.bass_utils import run_bass_kernel_spmd

F, M, C, OUT = 60, 600, 8, 18
B = 2048
NCORES = 8
BL = B // NCORES            # 256 batches per core
SB = 8                      # batches per superbatch
NSB = BL // SB              # 32 superbatches
FGB = 64                    # batches per finishing group
NFG = BL // FGB             # 4 finishing groups
SBPF = FGB // SB            # 8 superbatches per finishing group
CH = 5                      # chunks (of 120 rows) per batch
RP = M // CH                # 120 rows per chunk
PW = 608                    # padded per-batch-pair column length in xt2
NG = FGB * C                # 512 finishing columns

BF16 = mybir.dt.bfloat16
F32 = mybir.dt.float32
MULT = mybir.AluOpType.mult
EPS = 1e-12


def _build_nc():
    nc = bacc.Bacc(
        "TRN2", target_bir_lowering=False, debug=False,
        enable_asserts=False, num_devices=NCORES,
    )
    xg = nc.dram_tensor("xg", [NSB, RP, SB * CH * F], BF16,
                        kind="ExternalInput").ap()
    xt = nc.dram_tensor("xt", [NSB, 2 * F + 1, (SB // 2) * PW], BF16,
                        kind="ExternalInput").ap()
    wblk_d = nc.dram_tensor("wblk", [2 * F + 1, 2 * C], BF16,
                            kind="ExternalInput").ap()
    cst_d = nc.dram_tensor("cst", [128, 5 * C], F32, kind="ExternalInput").ap()
    hds_d = nc.dram_tensor("hds", [F, 2 * C * OUT], F32, kind="ExternalInput").ap()
    y = nc.dram_tensor("y", [BL, OUT], F32, kind="ExternalOutput").ap()

    with tile.TileContext(nc) as tc:
        _emit(tc, y, xg, xt, wblk_d, cst_d, hds_d)
    nc.compile()
    return nc


def _emit(tc, y, xg, xt, wblk_d, cst_d, hds_d):
    nc = tc.nc
    from contextlib import ExitStack
    ctx = ExitStack()
    with ctx:
        cpool = ctx.enter_context(tc.tile_pool(name="cpool", bufs=1))
        xpool = ctx.enter_context(tc.tile_pool(name="xpool", bufs=3))
        qpool = ctx.enter_context(tc.tile_pool(name="qpool", bufs=3))
        tpool = ctx.enter_context(tc.tile_pool(name="tpool", bufs=3))
        spool = ctx.enter_context(tc.tile_pool(name="spool", bufs=3))
        gpool = ctx.enter_context(tc.tile_pool(name="gpool", bufs=2))
        fpool = ctx.enter_context(tc.tile_pool(name="fpool", bufs=2))
        lpsum = ctx.enter_context(tc.tile_pool(name="lpsum", bufs=2, space="PSUM"))
        fpsum = ctx.enter_context(tc.tile_pool(name="fpsum", bufs=2, space="PSUM"))
        apsum = ctx.enter_context(tc.tile_pool(name="apsum", bufs=1, space="PSUM"))
        bpsum = ctx.enter_context(tc.tile_pool(name="bpsum", bufs=1, space="PSUM"))
        rpsum = ctx.enter_context(tc.tile_pool(name="rpsum", bufs=1, space="PSUM"))
        npsum = ctx.enter_context(tc.tile_pool(name="npsum", bufs=1, space="PSUM"))

        # ---- constants ----
        wblk = cpool.tile([2 * F + 1, 2 * C], BF16)
        nc.sync.dma_start(out=wblk[:], in_=wblk_d[:])
        cst = cpool.tile([128, 5 * C], F32)
        nc.sync.dma_start(out=cst[:], in_=cst_d[:])
        hds = cpool.tile([F, 2 * C * OUT], F32)
        nc.sync.dma_start(out=hds[:], in_=hds_d[:])
        k1 = cst[0:F, 0 * C:1 * C]
        w2k1 = cst[0:F, 1 * C:2 * C]
        bco64 = cst[64:64 + F, 2 * C:3 * C]   # used against stage[64:124]
        cco = cst[0:F, 3 * C:4 * C]
        dco = cst[0:F, 4 * C:5 * C]

        ones_r = cpool.tile([1, F], F32)   # lhsT for partition-broadcast
        nc.vector.memset(ones_r[:], 1.0)
        ones_c = cpool.tile([F, 1], F32)   # lhsT for partition-sum
        nc.vector.memset(ones_c[:], 1.0)
        eps1 = cpool.tile([1, 1], F32)     # l2-normalize epsilon
        nc.vector.memset(eps1[:], EPS)
        onem = cpool.tile([RP, 1], BF16)   # ones column: asum matmuls
        nc.vector.memset(onem[:], 1.0)

        def cb(ap):  # broadcast a [F, C] const across FGB batches
            return ap.unsqueeze(1).broadcast_to([F, FGB, C])

        for fg in range(NFG):
            stage = gpool.tile([128, NG], F32)
            asumst = gpool.tile([1, NG], F32)
            for s in range(SBPF):
                sb = fg * SBPF + s
                # ---- load superbatch (both DMAs fully contiguous) ----
                xgt = xpool.tile([RP, SB * CH * F], BF16)
                nc.sync.dma_start(out=xgt[:], in_=xg[sb])
                xtt = tpool.tile([2 * F + 1, (SB // 2) * PW], BF16)
                nc.sync.dma_start(out=xtt[:], in_=xt[sb])
                # ---- x^2 ----
                xsq = qpool.tile([RP, SB * CH * F], BF16)
                nc.vector.tensor_mul(xsq[:], xgt[:], xgt[:])
                # ---- logits: two batches per matmul via block-diag W ----
                lp = lpsum.tile([128, SB * CH * C], F32)
                for pr in range(SB // 2):
                    for c in range(CH):
                        nc.tensor.matmul(
                            lp[:, (pr * CH + c) * 2 * C:(pr * CH + c + 1) * 2 * C],
                            xtt[:, pr * PW + c * RP: pr * PW + c * RP + 128],
                            wblk[:],
                            start=True, stop=True,
                        )
                # ---- softmax over C ----
                expt = spool.tile([RP, SB * CH * C], F32, tag="expt")
                nc.scalar.activation(
                    expt[:], lp[0:RP, :], mybir.ActivationFunctionType.Exp
                )
                sums = spool.tile([RP, SB * CH], F32, tag="sums")
                nc.vector.reduce_sum(
                    out=sums[:],
                    in_=expt.rearrange("p (k e) -> p k e", e=C),
                    axis=mybir.AxisListType.X,
                )
                rin = spool.tile([RP, SB * CH], F32, tag="rin")
                nc.vector.reciprocal(rin[:], sums[:])
                actt = spool.tile([RP, SB * CH * C], BF16, tag="actt")
                nc.vector.tensor_tensor(
                    out=actt.rearrange("p (k e) -> p k e", e=C),
                    in0=expt.rearrange("p (k e) -> p k e", e=C),
                    in1=rin.unsqueeze(2).broadcast_to([RP, SB * CH, C]),
                    op=MULT,
                )
                # ---- fv accumulation (weight-stationary, 8-wide outs) ----
                fp = fpsum.tile([128, SB * C], F32)
                ap_ = apsum.tile([1, SB * C], F32)
                for b in range(SB):
                    pr, j = b // 2, b % 2
                    for c in range(CH):
                        a_sl = actt[:, (pr * CH + c) * 2 * C + j * C:
                                    (pr * CH + c) * 2 * C + (j + 1) * C]
                        nc.tensor.matmul(
                            fp[0:F, b * C:(b + 1) * C],
                            xgt[:, (b * CH + c) * F:(b * CH + c + 1) * F],
                            a_sl,
                            start=(c == 0), stop=(c == CH - 1),
                        )
                        nc.tensor.matmul(
                            fp[64:64 + F, b * C:(b + 1) * C],
                            xsq[:, (b * CH + c) * F:(b * CH + c + 1) * F],
                            a_sl,
                            start=(c == 0), stop=(c == CH - 1),
                        )
                        nc.tensor.matmul(
                            ap_[:, b * C:(b + 1) * C],
                            onem[:],
                            a_sl,
                            start=(c == 0), stop=(c == CH - 1),
                        )
                nc.vector.tensor_copy(
                    stage[:, s * SB * C:(s + 1) * SB * C], fp[:]
                )
                nc.scalar.copy(
                    asumst[:, s * SB * C:(s + 1) * SB * C], ap_[:]
                )

            # ---- finishing for this group of 64 batches ----
            fv1r = stage[0:F, :]
            fv2r = stage[64:64 + F, :]
            asb = bpsum.tile([F, NG], F32)
            nc.tensor.matmul(asb[:], ones_r[:], asumst[:], start=True, stop=True)

            t1 = fpool.tile([F, NG], F32, tag="t1")
            nc.vector.tensor_tensor(out=t1.rearrange("p (g e) -> p g e", e=C),
                                    in0=fv1r.rearrange("p (g e) -> p g e", e=C),
                                    in1=cb(k1), op=MULT)
            m1 = fpool.tile([F, NG], F32, tag="m1")
            nc.vector.tensor_tensor(out=m1.rearrange("p (g e) -> p g e", e=C),
                                    in0=asb.rearrange("p (g e) -> p g e", e=C),
                                    in1=cb(w2k1), op=MULT)
            fv1f = fpool.tile([F, NG], F32, tag="fv1f")
            nc.vector.tensor_sub(fv1f[:], t1[:], m1[:])
            q1 = fpool.tile([F, NG], F32, tag="q1")
            nc.vector.tensor_mul(q1[:], fv1f[:], fv1f[:])
            r1 = rpsum.tile([1, NG], F32, tag="rs")
            nc.tensor.matmul(r1[:], ones_c[:], q1[:], start=True, stop=True)
            sq1 = fpool.tile([1, NG], F32, tag="sq1")
            nc.scalar.activation(sq1[:], r1[:],
                                 mybir.ActivationFunctionType.Sqrt, bias=eps1[:])
            nr1 = fpool.tile([1, NG], F32, tag="nr1")
            nc.vector.reciprocal(nr1[:], sq1[:])
            nb1 = npsum.tile([F, NG], F32, tag="nb")
            nc.tensor.matmul(nb1[:], ones_r[:], nr1[:], start=True, stop=True)
            fv1n = fpool.tile([F, NG], F32, tag="fv1n")
            nc.vector.tensor_mul(fv1n[:], fv1f[:], nb1[:])

            u1 = fpool.tile([F, NG], F32, tag="u1")
            nc.vector.tensor_tensor(out=u1.rearrange("p (g e) -> p g e", e=C),
                                    in0=asb.rearrange("p (g e) -> p g e", e=C),
                                    in1=cb(dco), op=MULT)
            u2 = fpool.tile([F, NG], F32, tag="u2")
            nc.vector.tensor_tensor(out=u2.rearrange("p (g e) -> p g e", e=C),
                                    in0=fv2r.rearrange("p (g e) -> p g e", e=C),
                                    in1=bco64.unsqueeze(1).broadcast_to([F, FGB, C]),
                                    op=MULT)
            u3 = fpool.tile([F, NG], F32, tag="u3")
            nc.vector.tensor_add(u3[:], u1[:], u2[:])
            u4 = fpool.tile([F, NG], F32, tag="u4")
            nc.vector.tensor_tensor(out=u4.rearrange("p (g e) -> p g e", e=C),
                                    in0=fv1r.rearrange("p (g e) -> p g e", e=C),
                                    in1=cb(cco), op=MULT)
            fv2n = fpool.tile([F, NG], F32, tag="fv2n")
            nc.vector.tensor_sub(fv2n[:], u3[:], u4[:])
            q2 = fpool.tile([F, NG], F32, tag="q2")
            nc.vector.tensor_mul(q2[:], fv2n[:], fv2n[:])
            r2 = rpsum.tile([1, NG], F32, tag="rs")
            nc.tensor.matmul(r2[:], ones_c[:], q2[:], start=True, stop=True)
            r2c = fpool.tile([1, FGB], F32, tag="r2c")
            nc.vector.reduce_sum(out=r2c[:],
                                 in_=r2.rearrange("p (g e) -> p g e", e=C),
                                 axis=mybir.AxisListType.X)
            sq2 = fpool.tile([1, FGB], F32, tag="sq2")
            nc.scalar.activation(sq2[:], r2c[:],
                                 mybir.ActivationFunctionType.Sqrt, bias=eps1[:])
            nr2 = fpool.tile([1, FGB], F32, tag="nr2")
            nc.vector.reciprocal(nr2[:], sq2[:])
            nr2e = fpool.tile([1, NG], F32, tag="nr2e")
            nc.vector.tensor_copy(
                nr2e.rearrange("p (g e) -> p g e", e=C),
                nr2.unsqueeze(2).broadcast_to([1, FGB, C]),
            )
            nb2 = npsum.tile([F, NG], F32, tag="nb")
            nc.tensor.matmul(nb2[:], ones_r[:], nr2e[:], start=True, stop=True)
            fv2nn = fpool.tile([F, NG], F32, tag="fv2nn")
            nc.vector.tensor_mul(fv2nn[:], fv2n[:], nb2[:])

            # ---- head ----
            hp = rpsum.tile([FGB, OUT], F32, tag="rs")
            for ci in range(C):
                nc.tensor.matmul(
                    hp[:], fv1n[:, ci::C], hds[:, ci * OUT:(ci + 1) * OUT],
                    start=(ci == 0), stop=False,
                )
            for ci in range(C):
                nc.tensor.matmul(
                    hp[:], fv2nn[:, ci::C],
                    hds[:, (C + ci) * OUT:(C + ci + 1) * OUT],
                    start=False, stop=(ci == C - 1),
                )
            yt = fpool.tile([FGB, OUT], F32, tag="yt")
            nc.scalar.copy(yt[:], hp[:])
            nc.sync.dma_start(out=y[fg * FGB:(fg + 1) * FGB, :], in_=yt[:])


def _host_prep(reshaped_input, cluster_weights, covar_weights, cluster_biases,
               cluster_weights2, hidden1_weights):
    bf = ml_dtypes.bfloat16
    x = np.ascontiguousarray(reshaped_input, dtype=np.float32)
    xb = x.astype(bf)                                   # [B*M, F]
    # m-major: xgp[core][sb, p, (b*CH+c)*F + f]
    xgp = (xb.reshape(NCORES, NSB, SB * CH, RP, F)
             .transpose(0, 1, 3, 2, 4)
             .reshape(NCORES, NSB, RP, SB * CH * F))
    # f-major batch-pair packed: xtp[core][sb, r, pr*PW + m]
    x3 = xb.reshape(NCORES, NSB, SB // 2, 2, M, F)
    xtp = np.zeros((NCORES, NSB, 2 * F + 1, (SB // 2) * PW), dtype=bf)
    xtr = xtp.reshape(NCORES, NSB, 2 * F + 1, SB // 2, PW)
    xtr[:, :, 0:F, :, 0:M] = x3[:, :, :, 0].transpose(0, 1, 4, 2, 3)
    xtr[:, :, F:2 * F, :, 0:M] = x3[:, :, :, 1].transpose(0, 1, 4, 2, 3)
    xtr[:, :, 2 * F, :, :] = bf(1.0)

    wblk = np.zeros((2 * F + 1, 2 * C), dtype=bf)
    wblk[0:F, 0:C] = cluster_weights
    wblk[F:2 * F, C:2 * C] = cluster_weights
    wblk[2 * F, 0:C] = cluster_biases
    wblk[2 * F, C:2 * C] = cluster_biases

    cw = np.square(covar_weights.astype(np.float64)) + 1e-6       # [F, C]
    w2 = cluster_weights2[0].astype(np.float64)                   # [F, C]
    k1 = 1.0 / cw
    w2k1 = w2 / cw
    bcc = 1.0 / np.square(cw)
    ccc = 2.0 * w2 / np.square(cw)
    dcc = np.square(w2) / np.square(cw) - 1.0
    cst60 = np.concatenate([k1, w2k1, bcc, ccc, dcc], axis=1).astype(np.float32)
    cst = np.zeros((128, 5 * C), dtype=np.float32)
    cst[0:F] = cst60
    cst[64:64 + F] = cst60

    h = hidden1_weights.astype(np.float64)              # [2*C*F, OUT]
    h1 = h[:C * F].reshape(F, C, OUT) / math.sqrt(C)    # fold 2nd l2n of fv1
    h2 = h[C * F:].reshape(F, C, OUT)
    hds = np.concatenate([h1, h2], axis=1).reshape(F, 2 * C * OUT)
    hds = np.ascontiguousarray(hds, dtype=np.float32)

    in_maps = []
    for ci in range(NCORES):
        in_maps.append({
            "xg": np.ascontiguousarray(xgp[ci]),
            "xt": np.ascontiguousarray(xtp[ci]),
            "wblk": wblk,
            "cst": cst,
            "hds": hds,
        })
    return in_maps


_CACHE = {}


def _get_nc():
    if "nc" not in _CACHE:
        _CACHE["nc"] = _build_nc()
    return _CACHE["nc"]


def kernel(reshaped_input, cluster_weights, covar_weights, cluster_biases,
           cluster_weights2, hidden1_weights, **_kw):
    in_maps = _host_prep(reshaped_input, cluster_weights, covar_weights,
                         cluster_biases, cluster_weights2, hidden1_weights)
    nc = _get_nc()
    res = run_bass_kernel_spmd(nc, in_maps, list(range(NCORES)))
    ys = [res.results[ci]["y"] for ci in range(NCORES)]
    return np.ascontiguousarray(np.concatenate(ys, axis=0), dtype=np.float32)


if __name__ == "__main__":
    rng = np.random.default_rng(0)
    fake = {
        "reshaped_input": rng.standard_normal((B * M, F), dtype=np.float32),
        "cluster_weights": rng.standard_normal((F, C)).astype(np.float32) * 0.13,
        "covar_weights": rng.standard_normal((F, C)).astype(np.float32) * 0.13,
        "cluster_biases": rng.standard_normal((C,)).astype(np.float32) * 0.13,
        "cluster_weights2": rng.standard_normal((1, F, C)).astype(np.float32) * 0.13,
        "hidden1_weights": rng.standard_normal((2 * C * F, OUT)).astype(np.float32) * 0.35,
    }
    out = kernel(**fake)
    print("kernel output", out.shape, out.dtype, np.abs(out).mean())


# revision 24
# speedup vs baseline: 2.5255x; 1.0842x over previous
"""Trainium2 Bass kernel for nn_NetFV (NetFV pooling head).

Strategy (pure data parallel over 8 cores, 256 batches each):
  - Host: cast x to bf16 in two layouts, both DMA'd as fully contiguous
    ~4.8KB-per-partition lines (this is the whole ballgame: the kernel is
    HBM-bound and small descriptors halve-or-worse the DMA bus):
      xg  [NSB, 120, 2400]   m-major: row p = sample-within-chunk, cols
                             (b*5+c)*60+f for superbatch-batch b, chunk c.
      xt2 [NSB, 121, 2432]   f-major batch-PAIR packed: rows 0:60 = batch
                             even's 60 features, rows 60:120 = batch odd,
                             row 120 = ones (bias fold); cols pr*608+m with
                             m 600:608 zero-padded so every 128-wide matmul
                             window is in-bounds. 121/128 partitions vs the
                             naive 61/128.
  - Device, per superbatch of 8 batches:
      logits: 20 matmuls lhsT=xt2[121,128] window, rhs=block-diag W [121,16]
              -> psum [128, 16] (two batches at once)
      softmax: exp (Act engine), rowsum/recip/mul (DVE) on [120, 320]
      x^2: one DVE square [120, 2400] into its own tile
      fv: per batch per chunk, 3 accumulating weight-stationary matmuls
          (lhsT = x-chunk [120,60] -> psum rows 0:60; x^2-chunk -> rows
          64:124; ones [120,1] -> a_sum), each out free-size 8 (cheap).
  - Finishing per 64 batches, f-on-partitions [60, 512]: elementwise DVE ops
    with folded constants; partition reductions/broadcasts via tiny PE
    matmuls; second l2-normalize of fv1 folded into head weights; head as 16
    accumulated [60,64]x[60,18] matmuls.
"""

import math
import sys

for _p in ("/opt/trn_rl_repo", "/opt/pypackages"):
    if _p not in sys.path:
        sys.path.append(_p)

import ml_dtypes
import numpy as np

import concourse.bacc as bacc
import concourse.bass as bass
import concourse.mybir as mybir
import concourse.tile as tile
from concourse.bass_utils import run_bass_kernel_spmd

F, M, C, OUT = 60, 600, 8, 18
B = 2048
NCORES = 8
BL = B // NCORES            # 256 batches per core
SB = 8                      # batches per superbatch
NSB = BL // SB              # 32 superbatches
FGB = 64                    # batches per finishing group
NFG = BL // FGB             # 4 finishing groups
SBPF = FGB // SB            # 8 superbatches per finishing group
CH = 5                      # chunks (of 120 rows) per batch
RP = M // CH                # 120 rows per chunk
PW = 608                    # padded per-batch-pair column length in xt2
NG = FGB * C                # 512 finishing columns

BF16 = mybir.dt.bfloat16
F32 = mybir.dt.float32
MULT = mybir.AluOpType.mult
EPS = 1e-12


def _build_nc():
    nc = bacc.Bacc(
        "TRN2", target_bir_lowering=False, debug=False,
        enable_asserts=False, num_devices=NCORES,
    )
    xg = nc.dram_tensor("xg", [NSB, RP, SB * CH * F], BF16,
                        kind="ExternalInput").ap()
    xt = nc.dram_tensor("xt", [NSB, 2 * F + 1, (SB // 2) * PW], BF16,
                        kind="ExternalInput").ap()
    wblk_d = nc.dram_tensor("wblk", [2 * F + 1, 2 * C], BF16,
                            kind="ExternalInput").ap()
    cst_d = nc.dram_tensor("cst", [128, 5 * C], F32, kind="ExternalInput").ap()
    hds_d = nc.dram_tensor("hds", [F, 2 * C * OUT], F32, kind="ExternalInput").ap()
    y = nc.dram_tensor("y", [BL, OUT], F32, kind="ExternalOutput").ap()

    with tile.TileContext(nc) as tc:
        _emit(tc, y, xg, xt, wblk_d, cst_d, hds_d)
    nc.compile()
    return nc


def _emit(tc, y, xg, xt, wblk_d, cst_d, hds_d):
    nc = tc.nc
    from contextlib import ExitStack
    ctx = ExitStack()
    with ctx:
        cpool = ctx.enter_context(tc.tile_pool(name="cpool", bufs=1))
        xpool = ctx.enter_context(tc.tile_pool(name="xpool", bufs=3))
        ipool = ctx.enter_context(tc.tile_pool(name="ipool", bufs=3))
        tpool = ctx.enter_context(tc.tile_pool(name="tpool", bufs=3))
        spool = ctx.enter_context(tc.tile_pool(name="spool", bufs=3))
        gpool = ctx.enter_context(tc.tile_pool(name="gpool", bufs=2))
        fpool = ctx.enter_context(tc.tile_pool(name="fpool", bufs=2))
        lpsum = ctx.enter_context(tc.tile_pool(name="lpsum", bufs=2, space="PSUM"))
        fpsum = ctx.enter_context(tc.tile_pool(name="fpsum", bufs=2, space="PSUM"))
        apsum = ctx.enter_context(tc.tile_pool(name="apsum", bufs=1, space="PSUM"))
        bpsum = ctx.enter_context(tc.tile_pool(name="bpsum", bufs=1, space="PSUM"))
        rpsum = ctx.enter_context(tc.tile_pool(name="rpsum", bufs=1, space="PSUM"))
        npsum = ctx.enter_context(tc.tile_pool(name="npsum", bufs=1, space="PSUM"))

        # ---- constants ----
        wblk = cpool.tile([2 * F + 1, 2 * C], BF16)
        nc.sync.dma_start(out=wblk[:], in_=wblk_d[:])
        cst = cpool.tile([128, 5 * C], F32)
        nc.sync.dma_start(out=cst[:], in_=cst_d[:])
        hds = cpool.tile([F, 2 * C * OUT], F32)
        nc.sync.dma_start(out=hds[:], in_=hds_d[:])
        k1 = cst[0:F, 0 * C:1 * C]
        w2k1 = cst[0:F, 1 * C:2 * C]
        bco64 = cst[64:64 + F, 2 * C:3 * C]   # used against stage[64:124]
        cco = cst[0:F, 3 * C:4 * C]
        dco = cst[0:F, 4 * C:5 * C]

        ones_r = cpool.tile([1, F], F32)   # lhsT for partition-broadcast
        nc.vector.memset(ones_r[:], 1.0)
        ones_c = cpool.tile([F, 1], F32)   # lhsT for partition-sum
        nc.vector.memset(ones_c[:], 1.0)
        eps1 = cpool.tile([1, 1], F32)     # l2-normalize epsilon
        nc.vector.memset(eps1[:], EPS)
        onem = cpool.tile([RP, 1], BF16)   # ones column: asum matmuls
        nc.vector.memset(onem[:], 1.0)

        def cb(ap):  # broadcast a [F, C] const across FGB batches
            return ap.unsqueeze(1).broadcast_to([F, FGB, C])

        NCH = SB * CH                   # 40 chunk-slots per superbatch
        SW = 2 * F + 4                  # slot width: x | 4 junk | x^2 (64-align)

        for fg in range(NFG):
            stage = gpool.tile([128, NG], F32)
            asumst = gpool.tile([1, NG], F32)
            for s in range(SBPF):
                sb = fg * SBPF + s
                # ---- load superbatch (contiguous DMAs on two HW queues) ----
                xstg = xpool.tile([RP, NCH * F], BF16)
                nc.sync.dma_start(out=xstg[:], in_=xg[sb])
                xtt = tpool.tile([2 * F + 1, (SB // 2) * PW], BF16)
                nc.scalar.dma_start(out=xtt[:], in_=xt[sb])
                # ---- interleave [x|x^2] per chunk-slot so each fv matmul
                # loads one [120, 120] stationary tile. Copy split Act/Pool;
                # square on DVE. ----
                xfull = ipool.tile([RP, NCH * SW], BF16)
                xsv = xstg.rearrange("p (k f) -> p k f", f=F)
                xiv = xfull.rearrange("p (k w) -> p k w", w=SW)
                nc.scalar.copy(xiv[:, 0:NCH // 2, 0:F], xsv[:, 0:NCH // 2, :])
                nc.gpsimd.tensor_copy(xiv[:, NCH // 2:NCH, 0:F],
                                      xsv[:, NCH // 2:NCH, :])
                nc.vector.tensor_tensor(
                    out=xiv[:, :, 64:64 + F], in0=xsv[:], in1=xsv[:], op=MULT,
                )
                # ---- logits: two batches per matmul via block-diag W ----
                lp = lpsum.tile([128, SB * CH * C], F32)
                for pr in range(SB // 2):
                    for c in range(CH):
                        nc.tensor.matmul(
                            lp[:, (pr * CH + c) * 2 * C:(pr * CH + c + 1) * 2 * C],
                            xtt[:, pr * PW + c * RP: pr * PW + c * RP + 128],
                            wblk[:],
                            start=True, stop=True,
                        )
                # ---- softmax over C ----
                expt = spool.tile([RP, SB * CH * C], F32, tag="expt")
                nc.scalar.activation(
                    expt[:], lp[0:RP, :], mybir.ActivationFunctionType.Exp
                )
                sums = spool.tile([RP, SB * CH], F32, tag="sums")
                nc.vector.reduce_sum(
                    out=sums[:],
                    in_=expt.rearrange("p (k e) -> p k e", e=C),
                    axis=mybir.AxisListType.X,
                )
                rin = spool.tile([RP, SB * CH], F32, tag="rin")
                nc.vector.reciprocal(rin[:], sums[:])
                actt = spool.tile([RP, SB * CH * C], BF16, tag="actt")
                nc.vector.tensor_tensor(
                    out=actt.rearrange("p (k e) -> p k e", e=C),
                    in0=expt.rearrange("p (k e) -> p k e", e=C),
                    in1=rin.unsqueeze(2).broadcast_to([RP, SB * CH, C]),
                    op=MULT,
                )
                # ---- fv accumulation: one [x|junk|x^2] lhsT per batch-chunk;
                # fv1 lands at psum rows 0:60, fv2 at rows 64:124 ----
                fp = fpsum.tile([128, SB * C], F32)
                for b in range(SB):
                    pr, j = b // 2, b % 2
                    for c in range(CH):
                        a_sl = actt[:, (pr * CH + c) * 2 * C + j * C:
                                    (pr * CH + c) * 2 * C + (j + 1) * C]
                        nc.tensor.matmul(
                            fp[0:SW, b * C:(b + 1) * C],
                            xfull[:, (b * CH + c) * SW:
                                  (b * CH + c + 1) * SW],
                            a_sl,
                            start=(c == 0), stop=(c == CH - 1),
                        )
                # ---- a_sum: one ones-matmul, then reduce over chunks ----
                asp = apsum.tile([1, NCH * C], F32)
                nc.tensor.matmul(asp[:], onem[:], actt[:], start=True, stop=True)
                nc.vector.reduce_sum(
                    out=asumst[:, s * SB * C:(s + 1) * SB * C]
                        .rearrange("p (a q) -> p a q", q=2 * C),
                    in_=asp.rearrange("p (a c q) -> p a c q", c=CH, q=2 * C)
                        .transpose([0, 1, 3, 2]),
                    axis=mybir.AxisListType.X,
                )
                nc.scalar.copy(
                    stage[0:SW, s * SB * C:(s + 1) * SB * C], fp[0:SW, :]
                )

            # ---- finishing for this group of 64 batches ----
            fv1r = stage[0:F, :]
            fv2r = stage[64:64 + F, :]
            asb = bpsum.tile([F, NG], F32)
            nc.tensor.matmul(asb[:], ones_r[:], asumst[:], start=True, stop=True)

            t1 = fpool.tile([F, NG], F32, tag="t1")
            nc.vector.tensor_tensor(out=t1.rearrange("p (g e) -> p g e", e=C),
                                    in0=fv1r.rearrange("p (g e) -> p g e", e=C),
                                    in1=cb(k1), op=MULT)
            m1 = fpool.tile([F, NG], F32, tag="m1")
            nc.vector.tensor_tensor(out=m1.rearrange("p (g e) -> p g e", e=C),
                                    in0=asb.rearrange("p (g e) -> p g e", e=C),
                                    in1=cb(w2k1), op=MULT)
            fv1f = fpool.tile([F, NG], F32, tag="fv1f")
            nc.vector.tensor_sub(fv1f[:], t1[:], m1[:])
            q1 = fpool.tile([F, NG], F32, tag="q1")
            nc.vector.tensor_mul(q1[:], fv1f[:], fv1f[:])
            r1 = rpsum.tile([1, NG], F32, tag="rs")
            nc.tensor.matmul(r1[:], ones_c[:], q1[:], start=True, stop=True)
            sq1 = fpool.tile([1, NG], F32, tag="sq1")
            nc.scalar.activation(sq1[:], r1[:],
                                 mybir.ActivationFunctionType.Sqrt, bias=eps1[:])
            nr1 = fpool.tile([1, NG], F32, tag="nr1")
            nc.vector.reciprocal(nr1[:], sq1[:])
            nb1 = npsum.tile([F, NG], F32, tag="nb")
            nc.tensor.matmul(nb1[:], ones_r[:], nr1[:], start=True, stop=True)
            fv1n = fpool.tile([F, NG], F32, tag="fv1n")
            nc.vector.tensor_mul(fv1n[:], fv1f[:], nb1[:])

            u1 = fpool.tile([F, NG], F32, tag="u1")
            nc.vector.tensor_tensor(out=u1.rearrange("p (g e) -> p g e", e=C),
                                    in0=asb.rearrange("p (g e) -> p g e", e=C),
                                    in1=cb(dco), op=MULT)
            u2 = fpool.tile([F, NG], F32, tag="u2")
            nc.vector.tensor_tensor(out=u2.rearrange("p (g e) -> p g e", e=C),
                                    in0=fv2r.rearrange("p (g e) -> p g e", e=C),
                                    in1=bco64.unsqueeze(1).broadcast_to([F, FGB, C]),
                                    op=MULT)
            u3 = fpool.tile([F, NG], F32, tag="u3")
            nc.vector.tensor_add(u3[:], u1[:], u2[:])
            u4 = fpool.tile([F, NG], F32, tag="u4")
            nc.vector.tensor_tensor(out=u4.rearrange("p (g e) -> p g e", e=C),
                                    in0=fv1r.rearrange("p (g e) -> p g e", e=C),
                                    in1=cb(cco), op=MULT)
            fv2n = fpool.tile([F, NG], F32, tag="fv2n")
            nc.vector.tensor_sub(fv2n[:], u3[:], u4[:])
            q2 = fpool.tile([F, NG], F32, tag="q2")
            nc.vector.tensor_mul(q2[:], fv2n[:], fv2n[:])
            r2 = rpsum.tile([1, NG], F32, tag="rs")
            nc.tensor.matmul(r2[:], ones_c[:], q2[:], start=True, stop=True)
            r2c = fpool.tile([1, FGB], F32, tag="r2c")
            nc.vector.reduce_sum(out=r2c[:],
                                 in_=r2.rearrange("p (g e) -> p g e", e=C),
                                 axis=mybir.AxisListType.X)
            sq2 = fpool.tile([1, FGB], F32, tag="sq2")
            nc.scalar.activation(sq2[:], r2c[:],
                                 mybir.ActivationFunctionType.Sqrt, bias=eps1[:])
            nr2 = fpool.tile([1, FGB], F32, tag="nr2")
            nc.vector.reciprocal(nr2[:], sq2[:])
            nr2e = fpool.tile([1, NG], F32, tag="nr2e")
            nc.vector.tensor_copy(
                nr2e.rearrange("p (g e) -> p g e", e=C),
                nr2.unsqueeze(2).broadcast_to([1, FGB, C]),
            )
            nb2 = npsum.tile([F, NG], F32, tag="nb")
            nc.tensor.matmul(nb2[:], ones_r[:], nr2e[:], start=True, stop=True)
            fv2nn = fpool.tile([F, NG], F32, tag="fv2nn")
            nc.vector.tensor_mul(fv2nn[:], fv2n[:], nb2[:])

            # ---- head ----
            hp = rpsum.tile([FGB, OUT], F32, tag="rs")
            for ci in range(C):
                nc.tensor.matmul(
                    hp[:], fv1n[:, ci::C], hds[:, ci * OUT:(ci + 1) * OUT],
                    start=(ci == 0), stop=False,
                )
            for ci in range(C):
                nc.tensor.matmul(
                    hp[:], fv2nn[:, ci::C],
                    hds[:, (C + ci) * OUT:(C + ci + 1) * OUT],
                    start=False, stop=(ci == C - 1),
                )
            yt = fpool.tile([FGB, OUT], F32, tag="yt")
            nc.scalar.copy(yt[:], hp[:])
            nc.sync.dma_start(out=y[fg * FGB:(fg + 1) * FGB, :], in_=yt[:])


def _host_prep(reshaped_input, cluster_weights, covar_weights, cluster_biases,
               cluster_weights2, hidden1_weights):
    bf = ml_dtypes.bfloat16
    x = np.ascontiguousarray(reshaped_input, dtype=np.float32)
    xb = x.astype(bf)                                   # [B*M, F]
    # m-major: xgp[core][sb, p, (b*CH+c)*F + f]
    xgp = (xb.reshape(NCORES, NSB, SB * CH, RP, F)
             .transpose(0, 1, 3, 2, 4)
             .reshape(NCORES, NSB, RP, SB * CH * F))
    # f-major batch-pair packed: xtp[core][sb, r, pr*PW + m]
    x3 = xb.reshape(NCORES, NSB, SB // 2, 2, M, F)
    xtp = np.zeros((NCORES, NSB, 2 * F + 1, (SB // 2) * PW), dtype=bf)
    xtr = xtp.reshape(NCORES, NSB, 2 * F + 1, SB // 2, PW)
    xtr[:, :, 0:F, :, 0:M] = x3[:, :, :, 0].transpose(0, 1, 4, 2, 3)
    xtr[:, :, F:2 * F, :, 0:M] = x3[:, :, :, 1].transpose(0, 1, 4, 2, 3)
    xtr[:, :, 2 * F, :, :] = bf(1.0)

    wblk = np.zeros((2 * F + 1, 2 * C), dtype=bf)
    wblk[0:F, 0:C] = cluster_weights
    wblk[F:2 * F, C:2 * C] = cluster_weights
    wblk[2 * F, 0:C] = cluster_biases
    wblk[2 * F, C:2 * C] = cluster_biases

    cw = np.square(covar_weights.astype(np.float64)) + 1e-6       # [F, C]
    w2 = cluster_weights2[0].astype(np.float64)                   # [F, C]
    k1 = 1.0 / cw
    w2k1 = w2 / cw
    bcc = 1.0 / np.square(cw)
    ccc = 2.0 * w2 / np.square(cw)
    dcc = np.square(w2) / np.square(cw) - 1.0
    cst60 = np.concatenate([k1, w2k1, bcc, ccc, dcc], axis=1).astype(np.float32)
    cst = np.zeros((128, 5 * C), dtype=np.float32)
    cst[0:F] = cst60
    cst[64:64 + F] = cst60

    h = hidden1_weights.astype(np.float64)              # [2*C*F, OUT]
    h1 = h[:C * F].reshape(F, C, OUT) / math.sqrt(C)    # fold 2nd l2n of fv1
    h2 = h[C * F:].reshape(F, C, OUT)
    hds = np.concatenate([h1, h2], axis=1).reshape(F, 2 * C * OUT)
    hds = np.ascontiguousarray(hds, dtype=np.float32)

    in_maps = []
    for ci in range(NCORES):
        in_maps.append({
            "xg": np.ascontiguousarray(xgp[ci]),
            "xt": np.ascontiguousarray(xtp[ci]),
            "wblk": wblk,
            "cst": cst,
            "hds": hds,
        })
    return in_maps


_CACHE = {}


def _get_nc():
    if "nc" not in _CACHE:
        _CACHE["nc"] = _build_nc()
    return _CACHE["nc"]


def kernel(reshaped_input, cluster_weights, covar_weights, cluster_biases,
           cluster_weights2, hidden1_weights, **_kw):
    in_maps = _host_prep(reshaped_input, cluster_weights, covar_weights,
                         cluster_biases, cluster_weights2, hidden1_weights)
    nc = _get_nc()
    res = run_bass_kernel_spmd(nc, in_maps, list(range(NCORES)))
    ys = [res.results[ci]["y"] for ci in range(NCORES)]
    return np.ascontiguousarray(np.concatenate(ys, axis=0), dtype=np.float32)


if __name__ == "__main__":
    rng = np.random.default_rng(0)
    fake = {
        "reshaped_input": rng.standard_normal((B * M, F), dtype=np.float32),
        "cluster_weights": rng.standard_normal((F, C)).astype(np.float32) * 0.13,
        "covar_weights": rng.standard_normal((F, C)).astype(np.float32) * 0.13,
        "cluster_biases": rng.standard_normal((C,)).astype(np.float32) * 0.13,
        "cluster_weights2": rng.standard_normal((1, F, C)).astype(np.float32) * 0.13,
        "hidden1_weights": rng.standard_normal((2 * C * F, OUT)).astype(np.float32) * 0.35,
    }
    out = kernel(**fake)
    print("kernel output", out.shape, out.dtype, np.abs(out).mean())


# revision 29
# speedup vs baseline: 3.4372x; 1.3610x over previous
"""Trainium2 Bass kernel for nn_NetFV (NetFV pooling head).

Strategy (pure data parallel over 8 cores, 256 batches each):
  - Host: cast x to bf16 in two layouts, both DMA'd as fully contiguous
    ~4.8KB-per-partition lines (this is the whole ballgame: the kernel is
    HBM-bound and small descriptors halve-or-worse the DMA bus):
      xg  [NSB, 120, 2400]   m-major: row p = sample-within-chunk, cols
                             (b*5+c)*60+f for superbatch-batch b, chunk c.
      xt2 [NSB, 121, 2432]   f-major batch-PAIR packed: rows 0:60 = batch
                             even's 60 features, rows 60:120 = batch odd,
                             row 120 = ones (bias fold); cols pr*608+m with
                             m 600:608 zero-padded so every 128-wide matmul
                             window is in-bounds. 121/128 partitions vs the
                             naive 61/128.
  - Device, per superbatch of 8 batches:
      logits: 20 matmuls lhsT=xt2[121,128] window, rhs=block-diag W [121,16]
              -> psum [128, 16] (two batches at once)
      softmax: exp (Act engine), rowsum/recip/mul (DVE) on [120, 320]
      x^2: one DVE square [120, 2400] into its own tile
      fv: per batch per chunk, 3 accumulating weight-stationary matmuls
          (lhsT = x-chunk [120,60] -> psum rows 0:60; x^2-chunk -> rows
          64:124; ones [120,1] -> a_sum), each out free-size 8 (cheap).
  - Finishing per 64 batches, f-on-partitions [60, 512]: elementwise DVE ops
    with folded constants; partition reductions/broadcasts via tiny PE
    matmuls; second l2-normalize of fv1 folded into head weights; head as 16
    accumulated [60,64]x[60,18] matmuls.
"""

import math
import sys

for _p in ("/opt/trn_rl_repo", "/opt/pypackages"):
    if _p not in sys.path:
        sys.path.append(_p)

import ml_dtypes
import numpy as np

import concourse.bacc as bacc
import concourse.bass as bass
import concourse.mybir as mybir
import concourse.tile as tile
from concourse.bass_utils import run_bass_kernel_spmd

F, M, C, OUT = 60, 600, 8, 18
B = 2048
NCORES = 8
BL = B // NCORES            # 256 batches per core
SB = 8                      # batches per superbatch
NSB = BL // SB              # 32 superbatches
FGB = 64                    # batches per finishing group
NFG = BL // FGB             # 4 finishing groups
SBPF = FGB // SB            # 8 superbatches per finishing group
CH = 5                      # chunks (of 120 rows) per batch
RP = M // CH                # 120 rows per chunk
PW = 608                    # padded per-batch-pair column length in xt2
NG = FGB * C                # 512 finishing columns

BF16 = mybir.dt.bfloat16
F32 = mybir.dt.float32
MULT = mybir.AluOpType.mult
EPS = 1e-12


def _build_nc():
    nc = bacc.Bacc(
        "TRN2", target_bir_lowering=False, debug=False,
        enable_asserts=False, num_devices=NCORES,
    )
    xg = nc.dram_tensor("xg", [NSB, RP, SB * CH * 64], BF16,
                        kind="ExternalInput").ap()
    xt = nc.dram_tensor("xt", [NSB, 2 * F + 1, (SB // 2) * PW], BF16,
                        kind="ExternalInput").ap()
    wblk_d = nc.dram_tensor("wblk", [2 * F + 1, 2 * C], BF16,
                            kind="ExternalInput").ap()
    cst_d = nc.dram_tensor("cst", [128, 5 * C], F32, kind="ExternalInput").ap()
    hds_d = nc.dram_tensor("hds", [F, 2 * C * OUT], F32, kind="ExternalInput").ap()
    y = nc.dram_tensor("y", [BL, OUT], F32, kind="ExternalOutput").ap()

    with tile.TileContext(nc) as tc:
        _emit(tc, y, xg, xt, wblk_d, cst_d, hds_d)
    nc.compile()
    return nc


def _emit(tc, y, xg, xt, wblk_d, cst_d, hds_d):
    nc = tc.nc
    from contextlib import ExitStack
    ctx = ExitStack()
    with ctx:
        cpool = ctx.enter_context(tc.tile_pool(name="cpool", bufs=1))
        xpool = ctx.enter_context(tc.tile_pool(name="xpool", bufs=3))
        ipool = ctx.enter_context(tc.tile_pool(name="ipool", bufs=3))
        tpool = ctx.enter_context(tc.tile_pool(name="tpool", bufs=3))
        spool = ctx.enter_context(tc.tile_pool(name="spool", bufs=3))
        gpool = ctx.enter_context(tc.tile_pool(name="gpool", bufs=2))
        fpool = ctx.enter_context(tc.tile_pool(name="fpool", bufs=2))
        lpsum = ctx.enter_context(tc.tile_pool(name="lpsum", bufs=2, space="PSUM"))
        fpsum = ctx.enter_context(tc.tile_pool(name="fpsum", bufs=2, space="PSUM"))
        apsum = ctx.enter_context(tc.tile_pool(name="apsum", bufs=1, space="PSUM"))
        bpsum = ctx.enter_context(tc.tile_pool(name="bpsum", bufs=1, space="PSUM"))
        rpsum = ctx.enter_context(tc.tile_pool(name="rpsum", bufs=1, space="PSUM"))
        npsum = ctx.enter_context(tc.tile_pool(name="npsum", bufs=1, space="PSUM"))

        # ---- constants ----
        wblk = cpool.tile([2 * F + 1, 2 * C], BF16)
        nc.sync.dma_start(out=wblk[:], in_=wblk_d[:])
        cst = cpool.tile([128, 5 * C], F32)
        nc.sync.dma_start(out=cst[:], in_=cst_d[:])
        hds = cpool.tile([F, 2 * C * OUT], F32)
        nc.sync.dma_start(out=hds[:], in_=hds_d[:])
        k1 = cst[0:F, 0 * C:1 * C]
        w2k1 = cst[0:F, 1 * C:2 * C]
        bco64 = cst[64:64 + F, 2 * C:3 * C]   # used against stage[64:124]
        cco = cst[0:F, 3 * C:4 * C]
        dco = cst[0:F, 4 * C:5 * C]

        ones_r = cpool.tile([1, F], F32)   # lhsT for partition-broadcast
        nc.vector.memset(ones_r[:], 1.0)
        ones_c = cpool.tile([F, 1], F32)   # lhsT for partition-sum
        nc.vector.memset(ones_c[:], 1.0)
        eps1 = cpool.tile([1, 1], F32)     # l2-normalize epsilon
        nc.vector.memset(eps1[:], EPS)
        onem = cpool.tile([RP, 1], BF16)   # ones column: asum matmuls
        nc.vector.memset(onem[:], 1.0)

        def cb(ap):  # broadcast a [F, C] const across FGB batches
            return ap.unsqueeze(1).broadcast_to([F, FGB, C])

        NCH = SB * CH                   # 40 chunk-slots per superbatch

        for fg in range(NFG):
            stage = gpool.tile([128, NG], F32)
            asumst = gpool.tile([1, NG], F32)
            for s in range(SBPF):
                sb = fg * SBPF + s
                # ---- load superbatch (contiguous DMAs on two HW queues) ----
                # xstg slots are [x(60) | zeros(4)]: 64-wide for fast
                # LDWEIGHTS and 64-aligned psum row placement.
                xstg = xpool.tile([RP, NCH * 64], BF16)
                nc.sync.dma_start(out=xstg[:], in_=xg[sb])
                xtt = tpool.tile([2 * F + 1, (SB // 2) * PW], BF16)
                nc.scalar.dma_start(out=xtt[:], in_=xt[sb])
                # ---- x^2: one contiguous DVE square (zeros stay zero) ----
                xsq = ipool.tile([RP, NCH * 64], BF16)
                nc.vector.tensor_mul(xsq[:], xstg[:], xstg[:])
                # ---- logits: two batches per matmul via block-diag W ----
                lp = lpsum.tile([128, SB * CH * C], F32)
                for pr in range(SB // 2):
                    for c in range(CH):
                        nc.tensor.matmul(
                            lp[:, (pr * CH + c) * 2 * C:(pr * CH + c + 1) * 2 * C],
                            xtt[:, pr * PW + c * RP: pr * PW + c * RP + 128],
                            wblk[:],
                            start=True, stop=True,
                        )
                # ---- softmax over C ----
                expt = spool.tile([RP, SB * CH * C], F32, tag="expt")
                nc.scalar.activation(
                    expt[:], lp[0:RP, :], mybir.ActivationFunctionType.Exp
                )
                sums = spool.tile([RP, SB * CH], F32, tag="sums")
                nc.vector.reduce_sum(
                    out=sums[:],
                    in_=expt.rearrange("p (k e) -> p k e", e=C),
                    axis=mybir.AxisListType.X,
                )
                rin = spool.tile([RP, SB * CH], F32, tag="rin")
                nc.vector.reciprocal(rin[:], sums[:])
                actt = spool.tile([RP, SB * CH * C], BF16, tag="actt")
                nc.vector.tensor_tensor(
                    out=actt.rearrange("p (k e) -> p k e", e=C),
                    in0=expt.rearrange("p (k e) -> p k e", e=C),
                    in1=rin.unsqueeze(2).broadcast_to([RP, SB * CH, C]),
                    op=MULT,
                )
                # ---- fv accumulation: per batch-chunk, x-slot and x^2-slot
                # matmuls against the batch-PAIR act (N=16, 32B-aligned rhs);
                # fv1 at psum rows 0:60, fv2 at rows 64:124; per-batch [128,16]
                # psum region (only this batch's 8 columns are meaningful) ----
                fp = fpsum.tile([128, SB * 2 * C], F32)
                for b in range(SB):
                    pr = b // 2
                    for c in range(CH):
                        a_pr = actt[:, (pr * CH + c) * 2 * C:
                                    (pr * CH + c + 1) * 2 * C]
                        k = (b * CH + c) * 64
                        nc.tensor.matmul(
                            fp[0:64, b * 2 * C:(b + 1) * 2 * C],
                            xstg[:, k:k + 64], a_pr,
                            start=(c == 0), stop=(c == CH - 1),
                        )
                        nc.tensor.matmul(
                            fp[64:128, b * 2 * C:(b + 1) * 2 * C],
                            xsq[:, k:k + 64], a_pr,
                            start=(c == 0), stop=(c == CH - 1),
                        )
                # ---- a_sum: one ones-matmul, then reduce over chunks ----
                asp = apsum.tile([1, NCH * C], F32)
                nc.tensor.matmul(asp[:], onem[:], actt[:], start=True, stop=True)
                nc.vector.reduce_sum(
                    out=asumst[:, s * SB * C:(s + 1) * SB * C]
                        .rearrange("p (a q) -> p a q", q=2 * C),
                    in_=asp.rearrange("p (a c q) -> p a c q", c=CH, q=2 * C)
                        .transpose([0, 1, 3, 2]),
                    axis=mybir.AxisListType.X,
                )
                # extract each batch's real 8 columns from its [128,16] region
                fpv = fp.rearrange("p (e o j q) -> p e o j q", o=2, j=2, q=C)
                stv = stage[:, s * SB * C:(s + 1) * SB * C] \
                    .rearrange("p (e o q) -> p e o q", o=2, q=C)
                nc.scalar.copy(stv[:, :, 0, :], fpv[:, :, 0, 0, :])
                nc.scalar.copy(stv[:, :, 1, :], fpv[:, :, 1, 1, :])

            # ---- finishing for this group of 64 batches ----
            fv1r = stage[0:F, :]
            fv2r = stage[64:64 + F, :]
            asb = bpsum.tile([F, NG], F32)
            nc.tensor.matmul(asb[:], ones_r[:], asumst[:], start=True, stop=True)

            t1 = fpool.tile([F, NG], F32, tag="t1")
            nc.vector.tensor_tensor(out=t1.rearrange("p (g e) -> p g e", e=C),
                                    in0=fv1r.rearrange("p (g e) -> p g e", e=C),
                                    in1=cb(k1), op=MULT)
            m1 = fpool.tile([F, NG], F32, tag="m1")
            nc.vector.tensor_tensor(out=m1.rearrange("p (g e) -> p g e", e=C),
                                    in0=asb.rearrange("p (g e) -> p g e", e=C),
                                    in1=cb(w2k1), op=MULT)
            fv1f = fpool.tile([F, NG], F32, tag="fv1f")
            nc.vector.tensor_sub(fv1f[:], t1[:], m1[:])
            q1 = fpool.tile([F, NG], F32, tag="q1")
            nc.vector.tensor_mul(q1[:], fv1f[:], fv1f[:])
            r1 = rpsum.tile([1, NG], F32, tag="rs")
            nc.tensor.matmul(r1[:], ones_c[:], q1[:], start=True, stop=True)
            sq1 = fpool.tile([1, NG], F32, tag="sq1")
            nc.scalar.activation(sq1[:], r1[:],
                                 mybir.ActivationFunctionType.Sqrt, bias=eps1[:])
            nr1 = fpool.tile([1, NG], F32, tag="nr1")
            nc.vector.reciprocal(nr1[:], sq1[:])
            nb1 = npsum.tile([F, NG], F32, tag="nb")
            nc.tensor.matmul(nb1[:], ones_r[:], nr1[:], start=True, stop=True)
            fv1n = fpool.tile([F, NG], F32, tag="fv1n")
            nc.vector.tensor_mul(fv1n[:], fv1f[:], nb1[:])

            u1 = fpool.tile([F, NG], F32, tag="u1")
            nc.vector.tensor_tensor(out=u1.rearrange("p (g e) -> p g e", e=C),
                                    in0=asb.rearrange("p (g e) -> p g e", e=C),
                                    in1=cb(dco), op=MULT)
            u2 = fpool.tile([F, NG], F32, tag="u2")
            nc.vector.tensor_tensor(out=u2.rearrange("p (g e) -> p g e", e=C),
                                    in0=fv2r.rearrange("p (g e) -> p g e", e=C),
                                    in1=bco64.unsqueeze(1).broadcast_to([F, FGB, C]),
                                    op=MULT)
            u3 = fpool.tile([F, NG], F32, tag="u3")
            nc.vector.tensor_add(u3[:], u1[:], u2[:])
            u4 = fpool.tile([F, NG], F32, tag="u4")
            nc.vector.tensor_tensor(out=u4.rearrange("p (g e) -> p g e", e=C),
                                    in0=fv1r.rearrange("p (g e) -> p g e", e=C),
                                    in1=cb(cco), op=MULT)
            fv2n = fpool.tile([F, NG], F32, tag="fv2n")
            nc.vector.tensor_sub(fv2n[:], u3[:], u4[:])
            q2 = fpool.tile([F, NG], F32, tag="q2")
            nc.vector.tensor_mul(q2[:], fv2n[:], fv2n[:])
            r2 = rpsum.tile([1, NG], F32, tag="rs")
            nc.tensor.matmul(r2[:], ones_c[:], q2[:], start=True, stop=True)
            r2c = fpool.tile([1, FGB], F32, tag="r2c")
            nc.vector.reduce_sum(out=r2c[:],
                                 in_=r2.rearrange("p (g e) -> p g e", e=C),
                                 axis=mybir.AxisListType.X)
            sq2 = fpool.tile([1, FGB], F32, tag="sq2")
            nc.scalar.activation(sq2[:], r2c[:],
                                 mybir.ActivationFunctionType.Sqrt, bias=eps1[:])
            nr2 = fpool.tile([1, FGB], F32, tag="nr2")
            nc.vector.reciprocal(nr2[:], sq2[:])
            nr2e = fpool.tile([1, NG], F32, tag="nr2e")
            nc.vector.tensor_copy(
                nr2e.rearrange("p (g e) -> p g e", e=C),
                nr2.unsqueeze(2).broadcast_to([1, FGB, C]),
            )
            nb2 = npsum.tile([F, NG], F32, tag="nb")
            nc.tensor.matmul(nb2[:], ones_r[:], nr2e[:], start=True, stop=True)
            fv2nn = fpool.tile([F, NG], F32, tag="fv2nn")
            nc.vector.tensor_mul(fv2nn[:], fv2n[:], nb2[:])

            # ---- head ----
            hp = rpsum.tile([FGB, OUT], F32, tag="rs")
            for ci in range(C):
                nc.tensor.matmul(
                    hp[:], fv1n[:, ci::C], hds[:, ci * OUT:(ci + 1) * OUT],
                    start=(ci == 0), stop=False,
                )
            for ci in range(C):
                nc.tensor.matmul(
                    hp[:], fv2nn[:, ci::C],
                    hds[:, (C + ci) * OUT:(C + ci + 1) * OUT],
                    start=False, stop=(ci == C - 1),
                )
            yt = fpool.tile([FGB, OUT], F32, tag="yt")
            nc.scalar.copy(yt[:], hp[:])
            nc.sync.dma_start(out=y[fg * FGB:(fg + 1) * FGB, :], in_=yt[:])


def _host_prep(reshaped_input, cluster_weights, covar_weights, cluster_biases,
               cluster_weights2, hidden1_weights):
    bf = ml_dtypes.bfloat16
    x = np.ascontiguousarray(reshaped_input, dtype=np.float32)
    xb = x.astype(bf)                                   # [B*M, F]
    # m-major 64-padded slots: xgp[core][sb, p, (b*CH+c)*64 + f], cols 60:64 = 0
    x5 = (xb.reshape(NCORES, NSB, SB * CH, RP, F)
            .transpose(0, 1, 3, 2, 4))          # [NC, NSB, 120, 40, 60]
    xgp = np.zeros((NCORES, NSB, RP, SB * CH, 64), dtype=bf)
    xgp[..., 0:F] = x5
    xgp = xgp.reshape(NCORES, NSB, RP, SB * CH * 64)
    # f-major batch-pair packed: xtp[core][sb, r, pr*PW + m]
    x3 = xb.reshape(NCORES, NSB, SB // 2, 2, M, F)
    xtp = np.zeros((NCORES, NSB, 2 * F + 1, (SB // 2) * PW), dtype=bf)
    xtr = xtp.reshape(NCORES, NSB, 2 * F + 1, SB // 2, PW)
    xtr[:, :, 0:F, :, 0:M] = x3[:, :, :, 0].transpose(0, 1, 4, 2, 3)
    xtr[:, :, F:2 * F, :, 0:M] = x3[:, :, :, 1].transpose(0, 1, 4, 2, 3)
    xtr[:, :, 2 * F, :, :] = bf(1.0)

    wblk = np.zeros((2 * F + 1, 2 * C), dtype=bf)
    wblk[0:F, 0:C] = cluster_weights
    wblk[F:2 * F, C:2 * C] = cluster_weights
    wblk[2 * F, 0:C] = cluster_biases
    wblk[2 * F, C:2 * C] = cluster_biases

    cw = np.square(covar_weights.astype(np.float64)) + 1e-6       # [F, C]
    w2 = cluster_weights2[0].astype(np.float64)                   # [F, C]
    k1 = 1.0 / cw
    w2k1 = w2 / cw
    bcc = 1.0 / np.square(cw)
    ccc = 2.0 * w2 / np.square(cw)
    dcc = np.square(w2) / np.square(cw) - 1.0
    cst60 = np.concatenate([k1, w2k1, bcc, ccc, dcc], axis=1).astype(np.float32)
    cst = np.zeros((128, 5 * C), dtype=np.float32)
    cst[0:F] = cst60
    cst[64:64 + F] = cst60

    h = hidden1_weights.astype(np.float64)              # [2*C*F, OUT]
    h1 = h[:C * F].reshape(F, C, OUT) / math.sqrt(C)    # fold 2nd l2n of fv1
    h2 = h[C * F:].reshape(F, C, OUT)
    hds = np.concatenate([h1, h2], axis=1).reshape(F, 2 * C * OUT)
    hds = np.ascontiguousarray(hds, dtype=np.float32)

    in_maps = []
    for ci in range(NCORES):
        in_maps.append({
            "xg": np.ascontiguousarray(xgp[ci]),
            "xt": np.ascontiguousarray(xtp[ci]),
            "wblk": wblk,
            "cst": cst,
            "hds": hds,
        })
    return in_maps


_CACHE = {}


def _get_nc():
    if "nc" not in _CACHE:
        _CACHE["nc"] = _build_nc()
    return _CACHE["nc"]


def kernel(reshaped_input, cluster_weights, covar_weights, cluster_biases,
           cluster_weights2, hidden1_weights, **_kw):
    in_maps = _host_prep(reshaped_input, cluster_weights, covar_weights,
                         cluster_biases, cluster_weights2, hidden1_weights)
    nc = _get_nc()
    res = run_bass_kernel_spmd(nc, in_maps, list(range(NCORES)))
    ys = [res.results[ci]["y"] for ci in range(NCORES)]
    return np.ascontiguousarray(np.concatenate(ys, axis=0), dtype=np.float32)


if __name__ == "__main__":
    rng = np.random.default_rng(0)
    fake = {
        "reshaped_input": rng.standard_normal((B * M, F), dtype=np.float32),
        "cluster_weights": rng.standard_normal((F, C)).astype(np.float32) * 0.13,
        "covar_weights": rng.standard_normal((F, C)).astype(np.float32) * 0.13,
        "cluster_biases": rng.standard_normal((C,)).astype(np.float32) * 0.13,
        "cluster_weights2": rng.standard_normal((1, F, C)).astype(np.float32) * 0.13,
        "hidden1_weights": rng.standard_normal((2 * C * F, OUT)).astype(np.float32) * 0.35,
    }
    out = kernel(**fake)
    print("kernel output", out.shape, out.dtype, np.abs(out).mean())


# revision 32
# speedup vs baseline: 3.5349x; 1.0284x over previous
"""Trainium2 Bass kernel for nn_NetFV (NetFV pooling head).

Strategy (pure data parallel over 8 cores, 256 batches each):
  - Host: cast x to bf16 in two layouts, both DMA'd as fully contiguous
    ~4.8KB-per-partition lines (this is the whole ballgame: the kernel is
    HBM-bound and small descriptors halve-or-worse the DMA bus):
      xg  [NSB, 120, 2400]   m-major: row p = sample-within-chunk, cols
                             (b*5+c)*60+f for superbatch-batch b, chunk c.
      xt2 [NSB, 121, 2432]   f-major batch-PAIR packed: rows 0:60 = batch
                             even's 60 features, rows 60:120 = batch odd,
                             row 120 = ones (bias fold); cols pr*608+m with
                             m 600:608 zero-padded so every 128-wide matmul
                             window is in-bounds. 121/128 partitions vs the
                             naive 61/128.
  - Device, per superbatch of 8 batches:
      logits: 20 matmuls lhsT=xt2[121,128] window, rhs=block-diag W [121,16]
              -> psum [128, 16] (two batches at once)
      softmax: exp (Act engine), rowsum/recip/mul (DVE) on [120, 320]
      x^2: one DVE square [120, 2400] into its own tile
      fv: per batch per chunk, 3 accumulating weight-stationary matmuls
          (lhsT = x-chunk [120,60] -> psum rows 0:60; x^2-chunk -> rows
          64:124; ones [120,1] -> a_sum), each out free-size 8 (cheap).
  - Finishing per 64 batches, f-on-partitions [60, 512]: elementwise DVE ops
    with folded constants; partition reductions/broadcasts via tiny PE
    matmuls; second l2-normalize of fv1 folded into head weights; head as 16
    accumulated [60,64]x[60,18] matmuls.
"""

import math
import sys

for _p in ("/opt/trn_rl_repo", "/opt/pypackages"):
    if _p not in sys.path:
        sys.path.append(_p)

import ml_dtypes
import numpy as np

import concourse.bacc as bacc
import concourse.bass as bass
import concourse.mybir as mybir
import concourse.tile as tile
from concourse.bass_utils import run_bass_kernel_spmd

F, M, C, OUT = 60, 600, 8, 18
B = 2048
NCORES = 8
BL = B // NCORES            # 256 batches per core
SB = 8                      # batches per superbatch
NSB = BL // SB              # 32 superbatches
FGB = 64                    # batches per finishing group
NFG = BL // FGB             # 4 finishing groups
SBPF = FGB // SB            # 8 superbatches per finishing group
CH = 5                      # chunks (of 120 rows) per batch
RP = M // CH                # 120 rows per chunk
PW = 608                    # padded per-batch-pair column length in xt2
NG = FGB * C                # 512 finishing columns

BF16 = mybir.dt.bfloat16
F32 = mybir.dt.float32
MULT = mybir.AluOpType.mult
EPS = 1e-12


def _build_nc():
    nc = bacc.Bacc(
        "TRN2", target_bir_lowering=False, debug=False,
        enable_asserts=False, num_devices=NCORES,
    )
    xg = nc.dram_tensor("xg", [NSB, RP, SB * CH * 64], BF16,
                        kind="ExternalInput").ap()
    xt = nc.dram_tensor("xt", [NSB, 2 * F + 1, (SB // 2) * PW], BF16,
                        kind="ExternalInput").ap()
    wblk_d = nc.dram_tensor("wblk", [2 * F + 1, 2 * C], BF16,
                            kind="ExternalInput").ap()
    cst_d = nc.dram_tensor("cst", [128, 5 * C], F32, kind="ExternalInput").ap()
    hds_d = nc.dram_tensor("hds", [F, 2 * C * OUT], F32, kind="ExternalInput").ap()
    y = nc.dram_tensor("y", [BL, OUT], F32, kind="ExternalOutput").ap()

    with tile.TileContext(nc) as tc:
        _emit(tc, y, xg, xt, wblk_d, cst_d, hds_d)
    nc.compile()
    return nc


def _emit(tc, y, xg, xt, wblk_d, cst_d, hds_d):
    nc = tc.nc
    from contextlib import ExitStack
    ctx = ExitStack()
    with ctx:
        cpool = ctx.enter_context(tc.tile_pool(name="cpool", bufs=1))
        xpool = ctx.enter_context(tc.tile_pool(name="xpool", bufs=4))
        ipool = ctx.enter_context(tc.tile_pool(name="ipool", bufs=3))
        tpool = ctx.enter_context(tc.tile_pool(name="tpool", bufs=4))
        spool = ctx.enter_context(tc.tile_pool(name="spool", bufs=3))
        gpool = ctx.enter_context(tc.tile_pool(name="gpool", bufs=2))
        fpool = ctx.enter_context(tc.tile_pool(name="fpool", bufs=2))
        lpsum = ctx.enter_context(tc.tile_pool(name="lpsum", bufs=2, space="PSUM"))
        fpsum = ctx.enter_context(tc.tile_pool(name="fpsum", bufs=2, space="PSUM"))
        apsum = ctx.enter_context(tc.tile_pool(name="apsum", bufs=1, space="PSUM"))
        bpsum = ctx.enter_context(tc.tile_pool(name="bpsum", bufs=1, space="PSUM"))
        rpsum = ctx.enter_context(tc.tile_pool(name="rpsum", bufs=1, space="PSUM"))
        npsum = ctx.enter_context(tc.tile_pool(name="npsum", bufs=1, space="PSUM"))

        # ---- constants ----
        wblk = cpool.tile([2 * F + 1, 2 * C], BF16)
        nc.sync.dma_start(out=wblk[:], in_=wblk_d[:])
        cst = cpool.tile([128, 5 * C], F32)
        nc.sync.dma_start(out=cst[:], in_=cst_d[:])
        hds = cpool.tile([F, 2 * C * OUT], F32)
        nc.sync.dma_start(out=hds[:], in_=hds_d[:])
        k1 = cst[0:F, 0 * C:1 * C]
        w2k1 = cst[0:F, 1 * C:2 * C]
        bco64 = cst[64:64 + F, 2 * C:3 * C]   # used against stage[64:124]
        cco = cst[0:F, 3 * C:4 * C]
        dco = cst[0:F, 4 * C:5 * C]

        ones_r = cpool.tile([1, F], F32)   # lhsT for partition-broadcast
        nc.vector.memset(ones_r[:], 1.0)
        ones_c = cpool.tile([F, 1], F32)   # lhsT for partition-sum
        nc.vector.memset(ones_c[:], 1.0)
        eps1 = cpool.tile([1, 1], F32)     # l2-normalize epsilon
        nc.vector.memset(eps1[:], EPS)
        onem = cpool.tile([RP, 1], BF16)   # ones column: asum matmuls
        nc.vector.memset(onem[:], 1.0)

        def cb(ap):  # broadcast a [F, C] const across FGB batches
            return ap.unsqueeze(1).broadcast_to([F, FGB, C])

        NCH = SB * CH                   # 40 chunk-slots per superbatch
        # round-robin input DMAs over the three HWDGE/SWDGE queues so the
        # shared DMA-engine pool stays fed despite per-instruction gaps
        qeng = [nc.sync, nc.scalar, nc.gpsimd]
        qi = [0]

        def dma_rr(out, in_):
            qeng[qi[0] % 3].dma_start(out=out, in_=in_)
            qi[0] += 1

        for fg in range(NFG):
            stage = gpool.tile([128, NG], F32)
            asumst = gpool.tile([1, NG], F32)
            for s in range(SBPF):
                sb = fg * SBPF + s
                # ---- load superbatch (contiguous DMAs on two HW queues) ----
                # xstg slots are [x(60) | zeros(4)]: 64-wide for fast
                # LDWEIGHTS and 64-aligned psum row placement.
                xstg = xpool.tile([RP, NCH * 64], BF16)
                dma_rr(xstg[:], xg[sb])
                xtt = tpool.tile([2 * F + 1, (SB // 2) * PW], BF16)
                dma_rr(xtt[:], xt[sb])
                # ---- x^2: one contiguous DVE square (zeros stay zero) ----
                xsq = ipool.tile([RP, NCH * 64], BF16)
                nc.vector.tensor_mul(xsq[:], xstg[:], xstg[:])
                # ---- logits: two batches per matmul via block-diag W ----
                lp = lpsum.tile([128, SB * CH * C], F32)
                for pr in range(SB // 2):
                    for c in range(CH):
                        nc.tensor.matmul(
                            lp[:, (pr * CH + c) * 2 * C:(pr * CH + c + 1) * 2 * C],
                            xtt[:, pr * PW + c * RP: pr * PW + c * RP + 128],
                            wblk[:],
                            start=True, stop=True,
                        )
                # ---- softmax over C ----
                expt = spool.tile([RP, SB * CH * C], F32, tag="expt")
                nc.scalar.activation(
                    expt[:], lp[0:RP, :], mybir.ActivationFunctionType.Exp
                )
                sums = spool.tile([RP, SB * CH], F32, tag="sums")
                nc.vector.reduce_sum(
                    out=sums[:],
                    in_=expt.rearrange("p (k e) -> p k e", e=C),
                    axis=mybir.AxisListType.X,
                )
                rin = spool.tile([RP, SB * CH], F32, tag="rin")
                nc.vector.reciprocal(rin[:], sums[:])
                actt = spool.tile([RP, SB * CH * C], BF16, tag="actt")
                nc.vector.tensor_tensor(
                    out=actt.rearrange("p (k e) -> p k e", e=C),
                    in0=expt.rearrange("p (k e) -> p k e", e=C),
                    in1=rin.unsqueeze(2).broadcast_to([RP, SB * CH, C]),
                    op=MULT,
                )
                # ---- fv accumulation: per batch-chunk, x-slot and x^2-slot
                # matmuls against the batch-PAIR act (N=16, 32B-aligned rhs);
                # fv1 at psum rows 0:60, fv2 at rows 64:124; per-batch [128,16]
                # psum region (only this batch's 8 columns are meaningful) ----
                fp = fpsum.tile([128, SB * 2 * C], F32)
                for b in range(SB):
                    pr = b // 2
                    for c in range(CH):
                        a_pr = actt[:, (pr * CH + c) * 2 * C:
                                    (pr * CH + c + 1) * 2 * C]
                        k = (b * CH + c) * 64
                        nc.tensor.matmul(
                            fp[0:64, b * 2 * C:(b + 1) * 2 * C],
                            xstg[:, k:k + 64], a_pr,
                            start=(c == 0), stop=(c == CH - 1),
                        )
                        nc.tensor.matmul(
                            fp[64:128, b * 2 * C:(b + 1) * 2 * C],
                            xsq[:, k:k + 64], a_pr,
                            start=(c == 0), stop=(c == CH - 1),
                        )
                # ---- a_sum: one ones-matmul, then reduce over chunks ----
                asp = apsum.tile([1, NCH * C], F32)
                nc.tensor.matmul(asp[:], onem[:], actt[:], start=True, stop=True)
                nc.vector.reduce_sum(
                    out=asumst[:, s * SB * C:(s + 1) * SB * C]
                        .rearrange("p (a q) -> p a q", q=2 * C),
                    in_=asp.rearrange("p (a c q) -> p a c q", c=CH, q=2 * C)
                        .transpose([0, 1, 3, 2]),
                    axis=mybir.AxisListType.X,
                )
                # extract each batch's real 8 columns from its [128,16] region
                fpv = fp.rearrange("p (e o j q) -> p e o j q", o=2, j=2, q=C)
                stv = stage[:, s * SB * C:(s + 1) * SB * C] \
                    .rearrange("p (e o q) -> p e o q", o=2, q=C)
                nc.scalar.copy(stv[:, :, 0, :], fpv[:, :, 0, 0, :])
                nc.scalar.copy(stv[:, :, 1, :], fpv[:, :, 1, 1, :])

            # ---- finishing for this group of 64 batches ----
            fv1r = stage[0:F, :]
            fv2r = stage[64:64 + F, :]
            asb = bpsum.tile([F, NG], F32)
            nc.tensor.matmul(asb[:], ones_r[:], asumst[:], start=True, stop=True)

            t1 = fpool.tile([F, NG], F32, tag="t1")
            nc.vector.tensor_tensor(out=t1.rearrange("p (g e) -> p g e", e=C),
                                    in0=fv1r.rearrange("p (g e) -> p g e", e=C),
                                    in1=cb(k1), op=MULT)
            m1 = fpool.tile([F, NG], F32, tag="m1")
            nc.vector.tensor_tensor(out=m1.rearrange("p (g e) -> p g e", e=C),
                                    in0=asb.rearrange("p (g e) -> p g e", e=C),
                                    in1=cb(w2k1), op=MULT)
            fv1f = fpool.tile([F, NG], F32, tag="fv1f")
            nc.vector.tensor_sub(fv1f[:], t1[:], m1[:])
            q1 = fpool.tile([F, NG], F32, tag="q1")
            nc.vector.tensor_mul(q1[:], fv1f[:], fv1f[:])
            r1 = rpsum.tile([1, NG], F32, tag="rs")
            nc.tensor.matmul(r1[:], ones_c[:], q1[:], start=True, stop=True)
            sq1 = fpool.tile([1, NG], F32, tag="sq1")
            nc.scalar.activation(sq1[:], r1[:],
                                 mybir.ActivationFunctionType.Sqrt, bias=eps1[:])
            nr1 = fpool.tile([1, NG], F32, tag="nr1")
            nc.vector.reciprocal(nr1[:], sq1[:])
            nb1 = npsum.tile([F, NG], F32, tag="nb")
            nc.tensor.matmul(nb1[:], ones_r[:], nr1[:], start=True, stop=True)
            fv1n = fpool.tile([F, NG], F32, tag="fv1n")
            nc.vector.tensor_mul(fv1n[:], fv1f[:], nb1[:])

            u1 = fpool.tile([F, NG], F32, tag="u1")
            nc.vector.tensor_tensor(out=u1.rearrange("p (g e) -> p g e", e=C),
                                    in0=asb.rearrange("p (g e) -> p g e", e=C),
                                    in1=cb(dco), op=MULT)
            u2 = fpool.tile([F, NG], F32, tag="u2")
            nc.vector.tensor_tensor(out=u2.rearrange("p (g e) -> p g e", e=C),
                                    in0=fv2r.rearrange("p (g e) -> p g e", e=C),
                                    in1=bco64.unsqueeze(1).broadcast_to([F, FGB, C]),
                                    op=MULT)
            u3 = fpool.tile([F, NG], F32, tag="u3")
            nc.vector.tensor_add(u3[:], u1[:], u2[:])
            u4 = fpool.tile([F, NG], F32, tag="u4")
            nc.vector.tensor_tensor(out=u4.rearrange("p (g e) -> p g e", e=C),
                                    in0=fv1r.rearrange("p (g e) -> p g e", e=C),
                                    in1=cb(cco), op=MULT)
            fv2n = fpool.tile([F, NG], F32, tag="fv2n")
            nc.vector.tensor_sub(fv2n[:], u3[:], u4[:])
            q2 = fpool.tile([F, NG], F32, tag="q2")
            nc.vector.tensor_mul(q2[:], fv2n[:], fv2n[:])
            r2 = rpsum.tile([1, NG], F32, tag="rs")
            nc.tensor.matmul(r2[:], ones_c[:], q2[:], start=True, stop=True)
            r2c = fpool.tile([1, FGB], F32, tag="r2c")
            nc.vector.reduce_sum(out=r2c[:],
                                 in_=r2.rearrange("p (g e) -> p g e", e=C),
                                 axis=mybir.AxisListType.X)
            sq2 = fpool.tile([1, FGB], F32, tag="sq2")
            nc.scalar.activation(sq2[:], r2c[:],
                                 mybir.ActivationFunctionType.Sqrt, bias=eps1[:])
            nr2 = fpool.tile([1, FGB], F32, tag="nr2")
            nc.vector.reciprocal(nr2[:], sq2[:])
            nr2e = fpool.tile([1, NG], F32, tag="nr2e")
            nc.vector.tensor_copy(
                nr2e.rearrange("p (g e) -> p g e", e=C),
                nr2.unsqueeze(2).broadcast_to([1, FGB, C]),
            )
            nb2 = npsum.tile([F, NG], F32, tag="nb")
            nc.tensor.matmul(nb2[:], ones_r[:], nr2e[:], start=True, stop=True)
            fv2nn = fpool.tile([F, NG], F32, tag="fv2nn")
            nc.vector.tensor_mul(fv2nn[:], fv2n[:], nb2[:])

            # ---- head ----
            hp = rpsum.tile([FGB, OUT], F32, tag="rs")
            for ci in range(C):
                nc.tensor.matmul(
                    hp[:], fv1n[:, ci::C], hds[:, ci * OUT:(ci + 1) * OUT],
                    start=(ci == 0), stop=False,
                )
            for ci in range(C):
                nc.tensor.matmul(
                    hp[:], fv2nn[:, ci::C],
                    hds[:, (C + ci) * OUT:(C + ci + 1) * OUT],
                    start=False, stop=(ci == C - 1),
                )
            yt = fpool.tile([FGB, OUT], F32, tag="yt")
            nc.scalar.copy(yt[:], hp[:])
            nc.sync.dma_start(out=y[fg * FGB:(fg + 1) * FGB, :], in_=yt[:])


def _host_prep(reshaped_input, cluster_weights, covar_weights, cluster_biases,
               cluster_weights2, hidden1_weights):
    bf = ml_dtypes.bfloat16
    x = np.ascontiguousarray(reshaped_input, dtype=np.float32)
    xb = x.astype(bf)                                   # [B*M, F]
    # m-major 64-padded slots: xgp[core][sb, p, (b*CH+c)*64 + f], cols 60:64 = 0
    x5 = (xb.reshape(NCORES, NSB, SB * CH, RP, F)
            .transpose(0, 1, 3, 2, 4))          # [NC, NSB, 120, 40, 60]
    xgp = np.zeros((NCORES, NSB, RP, SB * CH, 64), dtype=bf)
    xgp[..., 0:F] = x5
    xgp = xgp.reshape(NCORES, NSB, RP, SB * CH * 64)
    # f-major batch-pair packed: xtp[core][sb, r, pr*PW + m]
    x3 = xb.reshape(NCORES, NSB, SB // 2, 2, M, F)
    xtp = np.zeros((NCORES, NSB, 2 * F + 1, (SB // 2) * PW), dtype=bf)
    xtr = xtp.reshape(NCORES, NSB, 2 * F + 1, SB // 2, PW)
    xtr[:, :, 0:F, :, 0:M] = x3[:, :, :, 0].transpose(0, 1, 4, 2, 3)
    xtr[:, :, F:2 * F, :, 0:M] = x3[:, :, :, 1].transpose(0, 1, 4, 2, 3)
    xtr[:, :, 2 * F, :, :] = bf(1.0)

    wblk = np.zeros((2 * F + 1, 2 * C), dtype=bf)
    wblk[0:F, 0:C] = cluster_weights
    wblk[F:2 * F, C:2 * C] = cluster_weights
    wblk[2 * F, 0:C] = cluster_biases
    wblk[2 * F, C:2 * C] = cluster_biases

    cw = np.square(covar_weights.astype(np.float64)) + 1e-6       # [F, C]
    w2 = cluster_weights2[0].astype(np.float64)                   # [F, C]
    k1 = 1.0 / cw
    w2k1 = w2 / cw
    bcc = 1.0 / np.square(cw)
    ccc = 2.0 * w2 / np.square(cw)
    dcc = np.square(w2) / np.square(cw) - 1.0
    cst60 = np.concatenate([k1, w2k1, bcc, ccc, dcc], axis=1).astype(np.float32)
    cst = np.zeros((128, 5 * C), dtype=np.float32)
    cst[0:F] = cst60
    cst[64:64 + F] = cst60

    h = hidden1_weights.astype(np.float64)              # [2*C*F, OUT]
    h1 = h[:C * F].reshape(F, C, OUT) / math.sqrt(C)    # fold 2nd l2n of fv1
    h2 = h[C * F:].reshape(F, C, OUT)
    hds = np.concatenate([h1, h2], axis=1).reshape(F, 2 * C * OUT)
    hds = np.ascontiguousarray(hds, dtype=np.float32)

    in_maps = []
    for ci in range(NCORES):
        in_maps.append({
            "xg": np.ascontiguousarray(xgp[ci]),
            "xt": np.ascontiguousarray(xtp[ci]),
            "wblk": wblk,
            "cst": cst,
            "hds": hds,
        })
    return in_maps


_CACHE = {}


def _get_nc():
    if "nc" not in _CACHE:
        _CACHE["nc"] = _build_nc()
    return _CACHE["nc"]


def kernel(reshaped_input, cluster_weights, covar_weights, cluster_biases,
           cluster_weights2, hidden1_weights, **_kw):
    in_maps = _host_prep(reshaped_input, cluster_weights, covar_weights,
                         cluster_biases, cluster_weights2, hidden1_weights)
    nc = _get_nc()
    res = run_bass_kernel_spmd(nc, in_maps, list(range(NCORES)))
    ys = [res.results[ci]["y"] for ci in range(NCORES)]
    return np.ascontiguousarray(np.concatenate(ys, axis=0), dtype=np.float32)


if __name__ == "__main__":
    rng = np.random.default_rng(0)
    fake = {
        "reshaped_input": rng.standard_normal((B * M, F), dtype=np.float32),
        "cluster_weights": rng.standard_normal((F, C)).astype(np.float32) * 0.13,
        "covar_weights": rng.standard_normal((F, C)).astype(np.float32) * 0.13,
        "cluster_biases": rng.standard_normal((C,)).astype(np.float32) * 0.13,
        "cluster_weights2": rng.standard_normal((1, F, C)).astype(np.float32) * 0.13,
        "hidden1_weights": rng.standard_normal((2 * C * F, OUT)).astype(np.float32) * 0.35,
    }
    out = kernel(**fake)
    print("kernel output", out.shape, out.dtype, np.abs(out).mean())
